# revision 1
# baseline (speedup 1.0000x reference)
"""Trainium2 Bass kernel for nn_DisenGCNLayer (disentangled GCN layer).

Strategy (8 NeuronCores, zero inter-core communication):
  - The 4 routing iterations are fully local per *destination* node: h_src is the
    fixed normalized feature table, and h_dst[row] only touches the node's own
    state. So destination nodes are sharded across cores; each core owns 49
    tiles of 128 nodes and all edges pointing into them.
  - Per tile, edges live in 128-wide chunks (edge-major SBUF layout). Segment
    softmax needs no segment-max (scores are dots of unit vectors, |s| <= 1),
    and the softmax denominator is factored out of the weighted aggregation, so
    denominator + aggregation reduce to ONE PSUM-accumulated matmul per chunk
    against a 0/1 "onehot" (edge -> local node) matrix built on-device.
  - h_src gathers use dma_gather (int16 indices, so the table is split at row
    32768 into A/B halves); per-iteration h_dst expansion uses dma_gather from a
    128-row HBM bounce buffer with tile-local indices.
"""

import heapq

import numpy as np
import ml_dtypes

import concourse.bass as bass
import concourse.bacc as bacc
import concourse.mybir as mybir
import concourse.tile as tile
from concourse.bass_utils import run_bass_kernel_spmd

bf16 = ml_dtypes.bfloat16

# problem spec (hardcoded)
N_NODES = 50000
N_EDGES = 800000
F = 128
K = 8
D = 16
ITERS = 4

NCORES = 8
P = 128
TILES = 392                 # total node tiles
TPC = TILES // NCORES       # 49 tiles per core
NPC = TPC * P               # 6272 nodes per core
NPAD = TILES * P            # 50176
SPLIT = 32768               # int16 gather index limit split
BROWS = NPAD - SPLIT

PGRP = 4                    # phase-0 node-chunk group size

TRACE = False               # test.py sets kernel.TRACE = True for profiling
DEBUG_STAGE = 99            # bisect: 1=phase0, 2=+init/gathers, 3=+1 iter, 99=full
LAST_RESULTS = {}           # exec info stash for test.py


# --------------------------------------------------------------------------
# host-side preprocessing
# --------------------------------------------------------------------------

def _preprocess(edge_index):
    row = np.asarray(edge_index[0], dtype=np.int64).astype(np.int32)
    col = np.asarray(edge_index[1], dtype=np.int64).astype(np.int32)

    degA = np.bincount(row[col < SPLIT], minlength=NPAD).astype(np.int64)
    degB = np.bincount(row[col >= SPLIT], minlength=NPAD).astype(np.int64)
    deg = degA + degB

    # --- bin-pack nodes into 392 tiles of exactly 128 nodes, balancing edges
    order = np.argsort(-deg, kind="stable")
    heap = [(0, t) for t in range(TILES)]
    heapq.heapify(heap)
    tile_nodes = [[] for _ in range(TILES)]
    tile_load = np.zeros(TILES, np.int64)
    for n in order:
        while True:
            load, t = heapq.heappop(heap)
            if len(tile_nodes[t]) < P:
                break
        tile_nodes[t].append(n)
        tile_load[t] = load + deg[n]
        if len(tile_nodes[t]) < P:
            heapq.heappush(heap, (tile_load[t], t))
    tileA = np.array([degA[tile_nodes[t]].sum() for t in range(TILES)])
    tileB = np.array([degB[tile_nodes[t]].sum() for t in range(TILES)])

    # --- assign tiles to cores: snake by total load, then sort each core's
    # tiles by nA desc so slot j is similar across cores (slot chunk counts
    # are maxed across cores and must be program-uniform).
    t_order = np.argsort(-(tileA + tileB), kind="stable")
    core_tiles = [[] for _ in range(NCORES)]
    for i, t in enumerate(t_order):
        c = i % (2 * NCORES)
        c = c if c < NCORES else 2 * NCORES - 1 - c
        core_tiles[c].append(t)
    for c in range(NCORES):
        core_tiles[c].sort(key=lambda t: -tileA[t])

    cA = np.zeros(TPC, np.int64)
    cB = np.zeros(TPC, np.int64)

    # --- per-node tile assignment + local slot
    node_tile = np.empty(NPAD, np.int32)
    node_local = np.empty(NPAD, np.int32)
    for t in range(TILES):
        ids = np.sort(np.array(tile_nodes[t], np.int64))
        tile_nodes[t] = ids
        node_tile[ids] = t
        node_local[ids] = np.arange(P, dtype=np.int32)

    # --- group edges by (tile, A/B)
    isB = (col >= SPLIT).astype(np.int64)
    ekey = node_tile[row].astype(np.int64) * 2 + isB
    eorder = np.argsort(ekey, kind="stable")
    ekey_s = ekey[eorder]
    col_s = col[eorder]
    seg_s = node_local[row[eorder]]
    # start offset of each (tile, side) group
    bounds = np.searchsorted(ekey_s, np.arange(2 * TILES + 1))

    def side_arrays(t, side, S_slots):
        """Pair-aware slot assignment for one (tile, side).

        Each node's edges are padded to even count; consecutive padded pairs
        land at slots (p, 2k) and (p, 2k+1) so one 512B descriptor (doubled
        bounce row) serves both. Returns slot_col, slot_seg, pair_val.
        """
        a0, a1 = bounds[2 * t + side], bounds[2 * t + side + 1]
        segs = seg_s[a0:a1]
        cols = col_s[a0:a1] - (SPLIT if side else 0)
        o2 = np.argsort(segs, kind="stable")
        segs = segs[o2].astype(np.int64)
        cols = cols[o2].astype(np.int64)
        counts = np.bincount(segs, minlength=P)
        padc = counts + (counts & 1)
        poffs = np.concatenate([[0], np.cumsum(padc)])
        offs_ = np.concatenate([[0], np.cumsum(counts)])
        rank = np.arange(len(segs)) - offs_[segs]
        ppos = poffs[segs] + rank
        assert poffs[-1] <= S_slots, (poffs[-1], S_slots)
        q, h = ppos // 2, ppos % 2
        slots = 128 * (2 * (q // 128) + h) + (q % 128)
        slot_col = np.zeros(S_slots, np.int16)
        slot_seg = np.full(S_slots, -1.0, np.float32)
        slot_col[slots] = cols
        slot_seg[slots] = segs
        pair_val = np.zeros(S_slots // 2, np.int16)
        ev = (ppos % 2 == 0)
        pair_val[ppos[ev] // 2] = segs[ev]
        return slot_col, slot_seg, pair_val

    def padded_len(t, side):
        a0, a1 = bounds[2 * t + side], bounds[2 * t + side + 1]
        counts = np.bincount(seg_s[a0:a1], minlength=P)
        return int((counts + (counts & 1)).sum())

    # chunk counts per tile slot: even, maxed across cores
    for j in range(TPC):
        ca = cb = 0
        for c in range(NCORES):
            t = core_tiles[c][j]
            ca = max(ca, -(-padded_len(t, 0) // P))
            cb = max(cb, -(-padded_len(t, 1) // P))
        cA[j] = ca + (ca & 1)
        cB[j] = cb + (cb & 1)

    def wrap16(idx, nslots):
        a = np.zeros(nslots, np.int16)
        a[: len(idx)] = idx
        return np.tile(np.ascontiguousarray(a.reshape(-1, 16).T), (8, 1))

    def wrap128(vals, nslots):
        a = np.full(nslots, -1.0, np.float32)
        a[: len(vals)] = vals
        return np.ascontiguousarray(a.reshape(-1, P).T)

    # --- per-core meta tensor (identical layout across cores)
    metas = []
    tile_meta_off = []
    node_order = []  # per core: global ids in output-row order
    for c in range(NCORES):
        parts = []
        offs = []
        off = 0
        ids_order = []
        for j in range(TPC):
            t = core_tiles[c][j]
            ids = tile_nodes[t]
            ids_order.append(ids)
            SA, SB = int(cA[j]) * P, int(cB[j]) * P
            C = int(cA[j] + cB[j])
            colA, segA, pairA = side_arrays(t, 0, SA)
            colB, segB, pairB = side_arrays(t, 1, SB)
            ninit = ids.astype(np.int32)[:, None]
            idxA = wrap16(colA, SA)
            idxB = wrap16(colB, SB)
            seg = np.concatenate([wrap128(segA, SA), wrap128(segB, SB)], 1).astype(bf16)
            pairidx = wrap16(np.concatenate([pairA, pairB]), C * 64)
            mt = np.concatenate(
                [ninit.view(np.int16).reshape(P, 2),
                 seg.view(np.int16),
                 idxA, idxB, pairidx], 1)
            if mt.shape[1] % 2:
                mt = np.concatenate([mt, np.zeros((P, 1), np.int16)], 1)
            offs.append((off, C))
            off += mt.shape[1]
            parts.append(mt)
        metas.append(np.ascontiguousarray(np.concatenate(parts, 1)))
        tile_meta_off.append(offs)
        node_order.append(np.concatenate(ids_order))
    # layout identical across cores
    assert all(o == tile_meta_off[0] for o in tile_meta_off[1:])
    assert all(m.shape == metas[0].shape for m in metas)
    return metas, tile_meta_off[0], node_order, cA, cB


# --------------------------------------------------------------------------
# device program
# --------------------------------------------------------------------------

def _build(offs, cA, cB, with_bias, meta_w):
    f32, b16, i16, i32 = (mybir.dt.float32, mybir.dt.bfloat16,
                          mybir.dt.int16, mybir.dt.int32)
    nc = bacc.Bacc()
    xt_in = nc.declare_dram_parameter("xt", [P, NPAD], f32, isOutput=False)
    w_in = nc.declare_dram_parameter("w", [F, F], f32, isOutput=False)
    meta_in = nc.declare_dram_parameter("meta", [P, meta_w], i16, isOutput=False)
    const_in = nc.declare_dram_parameter("const", [P, 384], i16, isOutput=False)
    out = nc.declare_dram_parameter("out", [NPC, F], f32, isOutput=True)

    h16 = nc.dram_tensor("h16", [NPAD, F], b16)

    AF = mybir.ActivationFunctionType
    OP = mybir.AluOpType

    with tile.TileContext(nc) as tc:
        with (
            tc.tile_pool(name="const", bufs=1) as constp,
            tc.tile_pool(name="meta", bufs=4) as metap,
            tc.tile_pool(name="xin", bufs=4) as xinp,
            tc.tile_pool(name="ph0", bufs=4) as ph0p,
            tc.tile_pool(name="big", bufs=3) as bigp,
            tc.tile_pool(name="hdp", bufs=4) as hdp,
            tc.tile_pool(name="small", bufs=8) as smallp,
            tc.tile_pool(name="psum", bufs=4, space="PSUM") as psump,
            tc.tile_pool(name="dram", bufs=16, space="DRAM") as dramp,
        ):
            # ---- constants
            ct = constp.tile([P, 384], i16)
            nc.sync.dma_start(out=ct[:], in_=const_in[:])
            iota_t = ct[:, 0:128].bitcast(b16)          # [128,128] row iota
            bias_t = ct[:, 128:384].bitcast(f32)        # [128,128] bias rows
            wt = constp.tile([P, F], f32)
            nc.sync.dma_start(out=wt[:], in_=w_in[:])
            eps_t = constp.tile([P, 1], f32)
            nc.vector.memset(eps_t[:], 1e-12)

            # ---- phase 0: h = normalize_k(leaky_relu(x @ W + b)) for all nodes
            NG = NPAD // (P * PGRP)                     # 98 groups
            for gi in range(NG):
                xts = xinp.tile([P, PGRP * F], f32, tag="xts")
                nc.sync.dma_start(out=xts[:], in_=xt_in[:, gi * PGRP * P:(gi + 1) * PGRP * P])
                hl = ph0p.tile([P, PGRP, F], f32, tag="hl")
                for j in range(PGRP):
                    hp = psump.tile([P, F], f32, tag="hp")
                    nc.tensor.matmul(out=hp[:], lhsT=xts[:, j * P:(j + 1) * P], rhs=wt[:], start=True, stop=True)
                    hsb = xinp.tile([P, F], f32, tag="hsb")
                    nc.scalar.activation(out=hsb[:], in_=hp[:], func=AF.Copy)
                    if with_bias:
                        nc.vector.tensor_tensor(out=hsb[:], in0=hsb[:], in1=bias_t, op=OP.add)
                    sc = xinp.tile([P, F], f32, tag="sc")
                    nc.gpsimd.tensor_scalar_mul(sc[:], hsb[:], 0.01)
                    nc.vector.tensor_tensor(out=hl[:, j, :], in0=hsb[:], in1=sc[:], op=OP.max)
                sq = ph0p.tile([P, PGRP, F], f32, tag="sq")
                nc.vector.tensor_tensor(out=sq[:], in0=hl[:], in1=hl[:], op=OP.mult)
                ss = ph0p.tile([P, PGRP, K], f32, tag="ss")
                nc.vector.tensor_reduce(
                    out=ss[:], in_=sq[:].rearrange("p g (d k) -> p g k d", d=D),
                    axis=mybir.AxisListType.X, op=OP.add)
                sroot = ph0p.tile([P, PGRP, K], f32, tag="sroot")
                nc.scalar.activation(out=sroot[:], in_=ss[:], func=AF.Sqrt, bias=eps_t[:])
                rs = ph0p.tile([P, PGRP, K], f32, tag="rs")
                nc.vector.reciprocal(out=rs[:], in_=sroot[:])
                hn = ph0p.tile([P, PGRP, F], f32, tag="hn")
                nc.vector.tensor_tensor(
                    out=hn[:].rearrange("p g (d k) -> p g d k", d=D),
                    in0=hl[:].rearrange("p g (d k) -> p g d k", d=D),
                    in1=rs[:].unsqueeze(2).to_broadcast([P, PGRP, D, K]),
                    op=OP.mult)
                hn16 = ph0p.tile([P, PGRP, F], b16, tag="hn16")
                nc.vector.tensor_copy(out=hn16[:], in_=hn[:])
                r0 = gi * PGRP * P
                nc.sync.dma_start(
                    out=h16[r0:r0 + PGRP * P, :].rearrange("(g p) f -> p g f", p=P),
                    in_=hn16[:])

            if DEBUG_STAGE == 1:
                zt = smallp.tile([P, F], f32, tag="zt")
                nc.vector.memset(zt[:], 0.0)
                for j in range(TPC):
                    nc.sync.dma_start(out=out[j * P:(j + 1) * P, :], in_=zt[:])
            # ---- iterations: tiles interleaved in groups to keep engines busy
            G = 3
            for grp0 in (range(0, TPC, G) if DEBUG_STAGE != 1 else []):
                grp = list(range(grp0, min(grp0 + G, TPC)))
                st = {}
                for j in grp:
                    off, C = offs[j]
                    SA, SB = int(cA[j]), int(cB[j])
                    W_t = 2 + C + 8 * SA + 8 * SB + 4 * C
                    W_t += W_t % 2
                    mt = metap.tile([P, W_t], i16, tag="mt")
                    nc.sync.dma_start(out=mt[:], in_=meta_in[:, off:off + W_t])
                    o = 2
                    seg_t = mt[:, o:o + C].bitcast(b16); o += C
                    idxA_t = mt[:, o:o + 8 * SA]; o += 8 * SA
                    idxB_t = mt[:, o:o + 8 * SB]; o += 8 * SB
                    idxhd_t = mt[:, o:o + 4 * C]; o += 4 * C
                    ninit_t = mt[:, 0:2].bitcast(i32)

                    # tile init: gather this tile's h_normed rows (bf16), seed bounce
                    hnt = smallp.tile([P, F], b16, tag="hnt")
                    nc.gpsimd.indirect_dma_start(
                        out=hnt[:], out_offset=None, in_=h16[:],
                        in_offset=bass.IndirectOffsetOnAxis(ap=ninit_t[:, :1], axis=0))
                    hntd = smallp.tile([P, 2, F], b16, tag="hntd")
                    nc.vector.tensor_copy(out=hntd[:], in_=hnt[:].unsqueeze(1).to_broadcast([P, 2, F]))
                    bounce = dramp.tile([P, 2 * F], b16, tag="bounce")
                    nc.sync.dma_start(out=bounce[:], in_=hntd[:])

                    # gather h_src for all edges (once per tile, reused 4 iters)
                    g = bigp.tile([P, C, F], b16, tag="g")
                    if SA and DEBUG_STAGE >= 23:
                        nc.gpsimd.dma_gather(
                            out_ap=g[:, 0:SA, :], in_ap=h16[0:SPLIT, :], idxs_ap=idxA_t,
                            num_idxs=SA * P, num_idxs_reg=SA * P, elem_size=F, single_packet=False)
                    if SB and DEBUG_STAGE >= 24:
                        nc.gpsimd.dma_gather(
                            out_ap=g[:, SA:C, :], in_ap=h16[SPLIT:NPAD, :], idxs_ap=idxB_t,
                            num_idxs=SB * P, num_idxs_reg=SB * P, elem_size=F, single_packet=False)
                    onehot = bigp.tile([P, C, P], b16, tag="onehot")
                    if DEBUG_STAGE >= 25:
                        nc.vector.tensor_tensor(
                            out=onehot[:],
                            in0=seg_t.unsqueeze(-1).to_broadcast([P, C, P]),
                            in1=iota_t.unsqueeze(1).to_broadcast([P, C, P]),
                            op=OP.is_equal)
                    st[j] = dict(C=C, idxhd=idxhd_t, bounce=bounce, g=g, onehot=onehot,
                                 hnt=hnt)
                    if 21 <= DEBUG_STAGE <= 29:
                        dbg = smallp.tile([P, F], f32, tag="dbg")
                        src_dbg = g[:, 0, :] if DEBUG_STAGE >= 23 else hnt[:]
                        nc.vector.tensor_copy(out=dbg[:], in_=src_dbg)
                        nc.sync.dma_start(out=out[j * P:(j + 1) * P, :], in_=dbg[:])
                if 21 <= DEBUG_STAGE <= 29:
                    continue

                for it in range(1 if DEBUG_STAGE == 3 else ITERS):
                    last_it = (it == (0 if DEBUG_STAGE == 3 else ITERS - 1))
                    for j in grp:
                        C = st[j]["C"]
                        hdexp2 = hdp.tile([P, C // 2, 2 * F], b16, tag="hdexp")
                        nc.gpsimd.dma_gather(
                            out_ap=hdexp2[:], in_ap=st[j]["bounce"][:], idxs_ap=st[j]["idxhd"],
                            num_idxs=C * 64, num_idxs_reg=C * 64, elem_size=2 * F,
                            single_packet=False)
                        st[j]["hdexp"] = hdexp2
                    for j in grp:
                        C, g = st[j]["C"], st[j]["g"]
                        hdexp = st[j]["hdexp"][:].rearrange("p c2 (h f) -> p (c2 h) f", h=2)
                        prod = hdp.tile([P, C, F], b16, tag="prod")
                        nc.vector.tensor_tensor(out=prod[:], in0=g[:], in1=hdexp, op=OP.mult)
                        pv = prod[:].rearrange("p c (d k) -> p c d k", d=D)
                        t1 = hdp.tile([P, C, 8, K], b16, tag="t1")
                        nc.vector.tensor_tensor(out=t1[:], in0=pv[:, :, 0:8, :], in1=pv[:, :, 8:16, :], op=OP.add)
                        t2 = hdp.tile([P, C, 4, K], b16, tag="t2")
                        nc.vector.tensor_tensor(out=t2[:], in0=t1[:, :, 0:4, :], in1=t1[:, :, 4:8, :], op=OP.add)
                        t3 = hdp.tile([P, C, 2, K], b16, tag="t3")
                        nc.vector.tensor_tensor(out=t3[:], in0=t2[:, :, 0:2, :], in1=t2[:, :, 2:4, :], op=OP.add)
                        scores = hdp.tile([P, C, K], f32, tag="scores")
                        nc.vector.tensor_tensor(
                            out=scores[:].unsqueeze(2), in0=t3[:, :, 0:1, :], in1=t3[:, :, 1:2, :], op=OP.add)
                        combo = hdp.tile([P, C, K + F], b16, tag="combo")
                        nc.scalar.activation(out=combo[:, :, 0:K], in_=scores[:], func=AF.Exp)
                        nc.vector.tensor_tensor(
                            out=combo[:, :, K:K + F].rearrange("p c (d k) -> p c d k", d=D),
                            in0=g[:].rearrange("p c (d k) -> p c d k", d=D),
                            in1=combo[:, :, 0:K].unsqueeze(2).to_broadcast([P, C, D, K]),
                            op=OP.mult)
                        st[j]["combo"] = combo
                    for j in grp:
                        C, onehot, combo = st[j]["C"], st[j]["onehot"], st[j]["combo"]
                        da = psump.tile([P, K + F], f32, tag="da")
                        for c in range(C):
                            nc.tensor.matmul(out=da[:], lhsT=onehot[:, c, :], rhs=combo[:, c, :],
                                             start=(c == 0), stop=(c == C - 1))
                        st[j]["da"] = da
                    for j in grp:
                        C, da, hnt = st[j]["C"], st[j]["da"], st[j]["hnt"]
                        deps = smallp.tile([P, K], f32, tag="deps")
                        nc.vector.tensor_scalar_add(deps[:], da[:, 0:K], 1e-6)
                        rden = smallp.tile([P, K], f32, tag="rden")
                        nc.vector.reciprocal(out=rden[:], in_=deps[:])
                        attr = smallp.tile([P, F], f32, tag="attr")
                        nc.vector.tensor_tensor(
                            out=attr[:].rearrange("p (d k) -> p d k", d=D),
                            in0=da[:, K:K + F].rearrange("p (d k) -> p d k", d=D),
                            in1=rden[:].unsqueeze(1).to_broadcast([P, D, K]),
                            op=OP.mult)
                        nc.vector.tensor_tensor(out=attr[:], in0=attr[:], in1=hnt[:], op=OP.add)
                        sq2 = smallp.tile([P, F], f32, tag="sq2")
                        nc.vector.tensor_tensor(out=sq2[:], in0=attr[:], in1=attr[:], op=OP.mult)
                        ss2 = smallp.tile([P, K], f32, tag="ss2")
                        nc.vector.tensor_reduce(
                            out=ss2[:], in_=sq2[:].rearrange("p (d k) -> p k d", d=D),
                            axis=mybir.AxisListType.X, op=OP.add)
                        sroot2 = smallp.tile([P, K], f32, tag="sroot2")
                        nc.scalar.activation(out=sroot2[:], in_=ss2[:], func=AF.Sqrt, bias=eps_t[:])
                        rs2 = smallp.tile([P, K], f32, tag="rs2")
                        nc.vector.reciprocal(out=rs2[:], in_=sroot2[:])
                        hnew = smallp.tile([P, F], f32, tag="hnew")
                        nc.vector.tensor_tensor(
                            out=hnew[:].rearrange("p (d k) -> p d k", d=D),
                            in0=attr[:].rearrange("p (d k) -> p d k", d=D),
                            in1=rs2[:].unsqueeze(1).to_broadcast([P, D, K]),
                            op=OP.mult)
                        if not last_it:
                            hnew16 = smallp.tile([P, 2, F], b16, tag="hnew16")
                            nc.vector.tensor_copy(out=hnew16[:], in_=hnew[:].unsqueeze(1).to_broadcast([P, 2, F]))
                            bounce = dramp.tile([P, 2 * F], b16, tag="bounce")
                            nc.sync.dma_start(out=bounce[:], in_=hnew16[:])
                            st[j]["bounce"] = bounce
                        else:
                            nc.sync.dma_start(out=out[j * P:(j + 1) * P, :], in_=hnew[:])
    if not nc.is_finalized():
        nc.finalize()
    return nc


# --------------------------------------------------------------------------
# entry point
# --------------------------------------------------------------------------

def kernel(x, edge_index, weight, bias):
    x = np.asarray(x, dtype=np.float32)
    weight = np.asarray(weight, dtype=np.float32)
    bias = np.asarray(bias, dtype=np.float32)
    assert x.shape == (N_NODES, F) and edge_index.shape == (2, N_EDGES)

    metas, offs, node_order, cA, cB = _preprocess(edge_index)
    with_bias = bool(np.any(bias != 0))
    nc = _build(offs, cA, cB, with_bias, metas[0].shape[1])

    # device uses (d, k)-interleaved feature order: f' = d*K + k <-> f = k*D + d
    perm = np.array([k * D + d for d in range(D) for k in range(K)])
    xpad = np.zeros((NPAD, F), np.float32)
    xpad[:N_NODES] = x
    xt = np.ascontiguousarray(xpad.T)                       # [128, NPAD]
    wp = np.ascontiguousarray(weight[:, perm])
    const = np.zeros((P, 384), np.int16)
    const[:, 0:128] = np.tile(np.arange(P, dtype=bf16)[None, :], (P, 1)).view(np.int16)
    const[:, 128:384] = np.tile(bias[perm].astype(np.float32)[None, :], (P, 1)).view(np.int16)

    in_maps = [
        dict(xt=xt, w=wp, meta=metas[c], const=const) for c in range(NCORES)
    ]
    res = run_bass_kernel_spmd(nc, in_maps, core_ids=list(range(NCORES)), trace=TRACE)
    LAST_RESULTS["exec_time_ns"] = res.exec_time_ns
    LAST_RESULTS["trace"] = res.instructions_and_trace
    LAST_RESULTS["nc"] = nc
    LAST_RESULTS["in_maps"] = in_maps

    full = np.zeros((NPAD, F), np.float32)
    for c in range(NCORES):
        full[node_order[c][:, None], perm[None, :]] = res.results[c]["out"]
    return full[:N_NODES]



# revision 7
# speedup vs baseline: 1.7747x; 1.7747x over previous
"""Trainium2 Bass kernel for nn_DisenGCNLayer (disentangled GCN layer).

Strategy (8 NeuronCores, zero inter-core communication):
  - Destination nodes sharded across cores; each core owns 49 tiles of 128
    nodes and all edges pointing into them. Per tile, edges live in 128-wide
    chunks (edge-major SBUF layout).
  - h_src gathered once per tile via dma_gather (int16 indices, table split at
    row 32768 into A/B halves).
  - Per-iteration h_dst expansion is a PE matmul against a transposed 0/1
    onehotT (node -> edge slot) built on-device, NOT a DMA gather: the Q7
    descriptor generation for per-edge gathers (~9.4ns/row) was the baseline
    bottleneck. Expanded rows land in PSUM f32 and are copied to SBUF bf16 by
    the scalar engine so the DVE product runs in 2x mode.
  - Segment softmax needs no segment-max (scores are dots of unit vectors,
    |s| <= 1); denominator is factored out of the weighted aggregation, so
    denominator + aggregation reduce to ONE PSUM-accumulated matmul per chunk
    against the edge-major onehot.
  - All activation funcs (leaky_relu, exp, ln, square, copy) live in ONE act
    table set (natural_log_exp_and_others): rsqrt is computed as
    exp(-0.5*ln(x)) to avoid Sqrt (different table -> 1.3us reload per swap).
"""

import heapq

import numpy as np
import ml_dtypes

import concourse.bass as bass
import concourse.bacc as bacc
import concourse.mybir as mybir
import concourse.tile as tile
from concourse.bass_utils import run_bass_kernel_spmd

bf16 = ml_dtypes.bfloat16

# problem spec (hardcoded)
N_NODES = 50000
N_EDGES = 800000
F = 128
K = 8
D = 16
ITERS = 4

NCORES = 8
P = 128
TILES = 392                 # total node tiles
TPC = TILES // NCORES       # 49 tiles per core
NPC = TPC * P               # 6272 nodes per core
NPAD = TILES * P            # 50176
SPLIT = 32768               # int16 gather index limit split

PGRP = 4                    # phase-0 node-chunk group size
CB = 8                      # expansion matmul PSUM batch (chunks)

TRACE = False               # test.py sets kernel.TRACE = True for profiling
DEBUG_STAGE = 99            # bisect: 1=phase0 only, 99=full
LAST_RESULTS = {}           # exec info stash for test.py


# --------------------------------------------------------------------------
# host-side preprocessing
# --------------------------------------------------------------------------

def _preprocess(edge_index):
    row = np.asarray(edge_index[0], dtype=np.int64).astype(np.int32)
    col = np.asarray(edge_index[1], dtype=np.int64).astype(np.int32)

    degA = np.bincount(row[col < SPLIT], minlength=NPAD).astype(np.int64)
    degB = np.bincount(row[col >= SPLIT], minlength=NPAD).astype(np.int64)
    deg = degA + degB

    # --- bin-pack nodes into 392 tiles of exactly 128 nodes, balancing edges
    order = np.argsort(-deg, kind="stable")
    heap = [(0, t) for t in range(TILES)]
    heapq.heapify(heap)
    tile_nodes = [[] for _ in range(TILES)]
    tile_load = np.zeros(TILES, np.int64)
    for n in order:
        while True:
            load, t = heapq.heappop(heap)
            if len(tile_nodes[t]) < P:
                break
        tile_nodes[t].append(n)
        tile_load[t] = load + deg[n]
        if len(tile_nodes[t]) < P:
            heapq.heappush(heap, (tile_load[t], t))
    tileA = np.array([degA[tile_nodes[t]].sum() for t in range(TILES)])
    tileB = np.array([degB[tile_nodes[t]].sum() for t in range(TILES)])

    # --- assign tiles to cores: snake by total load, then sort each core's
    # tiles by nA desc so slot j is similar across cores (slot chunk counts
    # are maxed across cores and must be program-uniform).
    t_order = np.argsort(-(tileA + tileB), kind="stable")
    core_tiles = [[] for _ in range(NCORES)]
    for i, t in enumerate(t_order):
        c = i % (2 * NCORES)
        c = c if c < NCORES else 2 * NCORES - 1 - c
        core_tiles[c].append(t)
    for c in range(NCORES):
        core_tiles[c].sort(key=lambda t: -tileA[t])

    cA = np.zeros(TPC, np.int64)
    cB = np.zeros(TPC, np.int64)

    # --- per-node tile assignment + local slot
    node_tile = np.empty(NPAD, np.int32)
    node_local = np.empty(NPAD, np.int32)
    for t in range(TILES):
        ids = np.sort(np.array(tile_nodes[t], np.int64))
        tile_nodes[t] = ids
        node_tile[ids] = t
        node_local[ids] = np.arange(P, dtype=np.int32)

    # --- group edges by (tile, A/B)
    isB = (col >= SPLIT).astype(np.int64)
    ekey = node_tile[row].astype(np.int64) * 2 + isB
    eorder = np.argsort(ekey, kind="stable")
    ekey_s = ekey[eorder]
    col_s = col[eorder]
    seg_s = node_local[row[eorder]]
    # start offset of each (tile, side) group
    bounds = np.searchsorted(ekey_s, np.arange(2 * TILES + 1))

    def side_arrays(t, side, S_slots):
        """Slot assignment for one (tile, side): edges in seg order, slot s at
        (partition s%128, chunk s//128). Padded slots: col 0, seg -1."""
        a0, a1 = bounds[2 * t + side], bounds[2 * t + side + 1]
        segs = seg_s[a0:a1]
        cols = col_s[a0:a1] - (SPLIT if side else 0)
        o2 = np.argsort(segs, kind="stable")
        segs = segs[o2].astype(np.int64)
        cols = cols[o2].astype(np.int64)
        n = len(segs)
        assert n <= S_slots, (n, S_slots)
        slot_col = np.zeros(S_slots, np.int16)
        slot_seg = np.full(S_slots, -1.0, np.float32)
        slot_col[:n] = cols
        slot_seg[:n] = segs
        return slot_col, slot_seg

    def nedges(t, side):
        a0, a1 = bounds[2 * t + side], bounds[2 * t + side + 1]
        return int(a1 - a0)

    # chunk counts per tile slot, maxed across cores (program-uniform)
    for j in range(TPC):
        ca = cb = 0
        for c in range(NCORES):
            t = core_tiles[c][j]
            ca = max(ca, -(-nedges(t, 0) // P))
            cb = max(cb, -(-nedges(t, 1) // P))
        cA[j] = max(ca, 1)
        cB[j] = max(cb, 1)

    def wrap16(idx, nslots):
        a = np.zeros(nslots, np.int16)
        a[: len(idx)] = idx
        return np.tile(np.ascontiguousarray(a.reshape(-1, 16).T), (8, 1))

    def wrap128(vals, nslots):
        a = np.full(nslots, -1.0, np.float32)
        a[: len(vals)] = vals
        return np.ascontiguousarray(a.reshape(-1, P).T)

    # --- per-core meta tensor (identical layout across cores)
    metas = []
    tile_meta_off = []
    node_order = []  # per core: global ids in output-row order
    for c in range(NCORES):
        parts = []
        offs = []
        off = 0
        ids_order = []
        for j in range(TPC):
            t = core_tiles[c][j]
            ids = tile_nodes[t]
            ids_order.append(ids)
            SA, SB = int(cA[j]) * P, int(cB[j]) * P
            C = int(cA[j] + cB[j])
            colA, segA = side_arrays(t, 0, SA)
            colB, segB = side_arrays(t, 1, SB)
            ninit = ids.astype(np.int32)[:, None]
            idxA = wrap16(colA, SA)
            idxB = wrap16(colB, SB)
            seg = np.concatenate([wrap128(segA, SA), wrap128(segB, SB)], 1).astype(bf16)
            mt = np.concatenate(
                [ninit.view(np.int16).reshape(P, 2),
                 seg.view(np.int16),
                 idxA, idxB], 1)
            if mt.shape[1] % 2:
                mt = np.concatenate([mt, np.zeros((P, 1), np.int16)], 1)
            offs.append((off, C))
            off += mt.shape[1]
            parts.append(mt)
        metas.append(np.ascontiguousarray(np.concatenate(parts, 1)))
        tile_meta_off.append(offs)
        node_order.append(np.concatenate(ids_order))
    # layout identical across cores
    assert all(o == tile_meta_off[0] for o in tile_meta_off[1:])
    assert all(m.shape == metas[0].shape for m in metas)
    return metas, tile_meta_off[0], node_order, cA, cB


# --------------------------------------------------------------------------
# device program
# --------------------------------------------------------------------------

def _build(offs, cA, cB, with_bias, meta_w):
    f32, b16, i16, i32 = (mybir.dt.float32, mybir.dt.bfloat16,
                          mybir.dt.int16, mybir.dt.int32)
    nc = bacc.Bacc()
    xt_in = nc.declare_dram_parameter("xt", [P, NPAD], f32, isOutput=False)
    w_in = nc.declare_dram_parameter("w", [F, F], f32, isOutput=False)
    meta_in = nc.declare_dram_parameter("meta", [P, meta_w], i16, isOutput=False)
    const_in = nc.declare_dram_parameter("const", [P, 512], i16, isOutput=False)
    out = nc.declare_dram_parameter("out", [NPC, F], f32, isOutput=True)

    h16 = nc.dram_tensor("h16", [NPAD, F], b16)

    AF = mybir.ActivationFunctionType
    OP = mybir.AluOpType

    with tile.TileContext(nc) as tc:
        with (
            tc.tile_pool(name="const", bufs=1) as constp,
            tc.tile_pool(name="meta", bufs=4) as metap,
            tc.tile_pool(name="xin", bufs=4) as xinp,
            tc.tile_pool(name="ph0", bufs=4) as ph0p,
            tc.tile_pool(name="big", bufs=3) as bigp,
            tc.tile_pool(name="hdp", bufs=4) as hdp,
            tc.tile_pool(name="small", bufs=8) as smallp,
            tc.tile_pool(name="pswork", bufs=2, space="PSUM") as pswork,
            tc.tile_pool(name="psda", bufs=2, space="PSUM") as psdap,
        ):
            # ---- constants
            ct = constp.tile([P, 512], i16)
            nc.sync.dma_start(out=ct[:], in_=const_in[:])
            iota_t = ct[:, 0:128].bitcast(b16)           # [128,128] row iota
            ident_t = ct[:, 128:256].bitcast(b16)        # [128,128] identity
            bias_t = ct[:, 256:512].bitcast(f32)         # [128,128] bias rows
            wt = constp.tile([P, F], f32)
            nc.sync.dma_start(out=wt[:], in_=w_in[:])
            eps_t = constp.tile([P, 1], f32)
            nc.vector.memset(eps_t[:], 1e-12)

            # ---- phase 0: h = normalize_k(leaky_relu(x @ W + b)) for all nodes
            NG = NPAD // (P * PGRP)                      # 98 groups
            for gi in range(NG):
                xts = xinp.tile([P, PGRP * F], f32, tag="xts")
                nc.sync.dma_start(out=xts[:], in_=xt_in[:, gi * PGRP * P:(gi + 1) * PGRP * P])
                hl = ph0p.tile([P, PGRP, F], f32, tag="hl")
                for j in range(PGRP):
                    hp = pswork.tile([P, CB, F], f32, tag="pse")
                    nc.tensor.matmul(out=hp[:, 0, :], lhsT=xts[:, j * P:(j + 1) * P], rhs=wt[:], start=True, stop=True)
                    if with_bias:
                        hb = xinp.tile([P, F], f32, tag="hb")
                        nc.vector.tensor_tensor(out=hb[:], in0=hp[:, 0, :], in1=bias_t, op=OP.add)
                        nc.scalar.activation(out=hl[:, j, :], in_=hb[:], func=AF.Prelu, alpha=0.01)
                    else:
                        nc.scalar.activation(out=hl[:, j, :], in_=hp[:, 0, :], func=AF.Prelu, alpha=0.01)
                sq = ph0p.tile([P, PGRP, F], f32, tag="sq")
                nc.vector.tensor_tensor(out=sq[:], in0=hl[:], in1=hl[:], op=OP.mult)
                ss = ph0p.tile([P, PGRP, K], f32, tag="ss")
                nc.vector.tensor_reduce(
                    out=ss[:], in_=sq[:].rearrange("p g (d k) -> p g k d", d=D),
                    axis=mybir.AxisListType.X, op=OP.add)
                lnv = ph0p.tile([P, PGRP, K], f32, tag="lnv")
                nc.scalar.activation(out=lnv[:], in_=ss[:], func=AF.Ln, bias=eps_t[:])
                rs = ph0p.tile([P, PGRP, K], f32, tag="rs")
                nc.scalar.activation(out=rs[:], in_=lnv[:], func=AF.Exp, scale=-0.5)
                hn16 = ph0p.tile([P, PGRP, F], b16, tag="hn16")
                nc.vector.tensor_tensor(
                    out=hn16[:].rearrange("p g (d k) -> p g d k", d=D),
                    in0=hl[:].rearrange("p g (d k) -> p g d k", d=D),
                    in1=rs[:].unsqueeze(2).to_broadcast([P, PGRP, D, K]),
                    op=OP.mult)
                r0 = gi * PGRP * P
                nc.sync.dma_start(
                    out=h16[r0:r0 + PGRP * P, :].rearrange("(g p) f -> p g f", p=P),
                    in_=hn16[:])

            if DEBUG_STAGE == 1:
                zt = smallp.tile([P, F], f32, tag="zt")
                nc.vector.memset(zt[:], 0.0)
                for j in range(TPC):
                    nc.sync.dma_start(out=out[j * P:(j + 1) * P, :], in_=zt[:])

            # ---- iterations: tiles interleaved in groups to keep engines busy
            G = 3
            for grp0 in (range(0, TPC, G) if DEBUG_STAGE != 1 else []):
                grp = list(range(grp0, min(grp0 + G, TPC)))
                st = {}
                for j in grp:
                    off, C = offs[j]
                    SA, SB = int(cA[j]), int(cB[j])
                    W_t = 2 + C + 8 * (SA + SB) * 16 // 16
                    W_t = 2 + C + 8 * SA + 8 * SB
                    W_t += W_t % 2
                    mt = metap.tile([P, W_t], i16, tag="mt")
                    nc.sync.dma_start(out=mt[:], in_=meta_in[:, off:off + W_t])
                    o = 2
                    seg_t = mt[:, o:o + C].bitcast(b16); o += C
                    idxA_t = mt[:, o:o + 8 * SA]; o += 8 * SA
                    idxB_t = mt[:, o:o + 8 * SB]; o += 8 * SB
                    ninit_t = mt[:, 0:2].bitcast(i32)

                    # tile init: gather this tile's h_normed rows (bf16)
                    hnt = smallp.tile([P, F], b16, tag="hnt")
                    nc.gpsimd.indirect_dma_start(
                        out=hnt[:], out_offset=None, in_=h16[:],
                        in_offset=bass.IndirectOffsetOnAxis(ap=ninit_t[:, :1], axis=0))

                    # gather h_src for all edges (once per tile, reused 4 iters)
                    g = bigp.tile([P, C, F], b16, tag="g")
                    if SA:
                        nc.gpsimd.dma_gather(
                            out_ap=g[:, 0:SA, :], in_ap=h16[0:SPLIT, :], idxs_ap=idxA_t,
                            num_idxs=SA * P, num_idxs_reg=SA * P, elem_size=F, single_packet=False)
                    if SB:
                        nc.gpsimd.dma_gather(
                            out_ap=g[:, SA:C, :], in_ap=h16[SPLIT:NPAD, :], idxs_ap=idxB_t,
                            num_idxs=SB * P, num_idxs_reg=SB * P, elem_size=F, single_packet=False)

                    # edge-major onehot (for aggregation matmul)
                    onehot = bigp.tile([P, C, P], b16, tag="onehot")
                    nc.vector.tensor_tensor(
                        out=onehot[:],
                        in0=seg_t.unsqueeze(-1).to_broadcast([P, C, P]),
                        in1=iota_t.unsqueeze(1).to_broadcast([P, C, P]),
                        op=OP.is_equal)
                    # transposed onehot (for hd expansion matmul), via PE
                    onehotT = bigp.tile([P, C, P], b16, tag="onehotT")
                    for cb0 in range(0, C, CB):
                        nb = min(CB, C - cb0)
                        psTf = pswork.tile([P, CB, F], f32, tag="pse")
                        psT = psTf[:].bitcast(b16)
                        for c in range(nb):
                            nc.tensor.transpose(
                                out=psT[:, c, 0:P], in_=onehot[:, cb0 + c, :], identity=ident_t)
                        nc.scalar.activation(
                            out=onehotT[:, cb0:cb0 + nb, :], in_=psT[:, 0:nb, 0:P], func=AF.Copy)

                    st[j] = dict(C=C, g=g, onehot=onehot, onehotT=onehotT, hnt=hnt,
                                 hd=hnt)

                for it in range(ITERS):
                    last_it = it == ITERS - 1
                    # expansion + prod (per PSUM batch of CB chunks)
                    for j in grp:
                        C, g, onehotT, hd = (st[j][k] for k in ("C", "g", "onehotT", "hd"))
                        prod = hdp.tile([P, C, F], b16, tag="prod")
                        for cb0 in range(0, C, CB):
                            nb = min(CB, C - cb0)
                            pse = pswork.tile([P, CB, F], f32, tag="pse")
                            for c in range(nb):
                                nc.tensor.matmul(
                                    out=pse[:, c, :], lhsT=onehotT[:, cb0 + c, :], rhs=hd[:],
                                    start=True, stop=True)
                            hdx = hdp.tile([P, CB, F], b16, tag="hdx")
                            nc.scalar.activation(out=hdx[:, 0:nb, :], in_=pse[:, 0:nb, :], func=AF.Copy)
                            nc.vector.tensor_tensor(
                                out=prod[:, cb0:cb0 + nb, :], in0=g[:, cb0:cb0 + nb, :],
                                in1=hdx[:, 0:nb, :], op=OP.mult)
                        st[j]["prod"] = prod
                    # scores (reduce tree) + exp + weighted combo
                    for j in grp:
                        C, g, prod = st[j]["C"], st[j]["g"], st[j]["prod"]
                        pv = prod[:].rearrange("p c (d k) -> p c d k", d=D)
                        t1 = hdp.tile([P, C, 8, K], b16, tag="t1")
                        nc.vector.tensor_tensor(out=t1[:], in0=pv[:, :, 0:8, :], in1=pv[:, :, 8:16, :], op=OP.add)
                        t2 = hdp.tile([P, C, 4, K], b16, tag="t2")
                        nc.vector.tensor_tensor(out=t2[:], in0=t1[:, :, 0:4, :], in1=t1[:, :, 4:8, :], op=OP.add)
                        t3 = hdp.tile([P, C, 2, K], b16, tag="t3")
                        nc.vector.tensor_tensor(out=t3[:], in0=t2[:, :, 0:2, :], in1=t2[:, :, 2:4, :], op=OP.add)
                        scores = hdp.tile([P, C, K], f32, tag="scores")
                        nc.vector.tensor_tensor(
                            out=scores[:].unsqueeze(2), in0=t3[:, :, 0:1, :], in1=t3[:, :, 1:2, :], op=OP.add)
                        combo = hdp.tile([P, C, K + F], b16, tag="combo")
                        nc.scalar.activation(out=combo[:, :, 0:K], in_=scores[:], func=AF.Exp)
                        nc.vector.tensor_tensor(
                            out=combo[:, :, K:K + F].rearrange("p c (d k) -> p c d k", d=D),
                            in0=g[:].rearrange("p c (d k) -> p c d k", d=D),
                            in1=combo[:, :, 0:K].unsqueeze(2).to_broadcast([P, C, D, K]),
                            op=OP.mult)
                        st[j]["combo"] = combo
                    # aggregation matmul (PSUM-accumulated over chunks)
                    for j in grp:
                        C, onehot, combo = st[j]["C"], st[j]["onehot"], st[j]["combo"]
                        da = psdap.tile([P, K + F], f32, tag="da")
                        for c in range(C):
                            nc.tensor.matmul(out=da[:], lhsT=onehot[:, c, :], rhs=combo[:, c, :],
                                             start=(c == 0), stop=(c == C - 1))
                        st[j]["da"] = da
                    # epilogue: normalize, update hd
                    for j in grp:
                        C, da, hnt = st[j]["C"], st[j]["da"], st[j]["hnt"]
                        deps = smallp.tile([P, K], f32, tag="deps")
                        nc.vector.tensor_scalar_add(deps[:], da[:, 0:K], 1e-6)
                        rden = smallp.tile([P, K], f32, tag="rden")
                        nc.vector.reciprocal(out=rden[:], in_=deps[:])
                        attr = smallp.tile([P, F], f32, tag="attr")
                        nc.vector.tensor_tensor(
                            out=attr[:].rearrange("p (d k) -> p d k", d=D),
                            in0=da[:, K:K + F].rearrange("p (d k) -> p d k", d=D),
                            in1=rden[:].unsqueeze(1).to_broadcast([P, D, K]),
                            op=OP.mult)
                        nc.vector.tensor_tensor(out=attr[:], in0=attr[:], in1=hnt[:], op=OP.add)
                        sq2 = smallp.tile([P, F], f32, tag="sq2")
                        nc.vector.tensor_tensor(out=sq2[:], in0=attr[:], in1=attr[:], op=OP.mult)
                        ss2 = smallp.tile([P, K], f32, tag="ss2")
                        nc.vector.tensor_reduce(
                            out=ss2[:], in_=sq2[:].rearrange("p (d k) -> p k d", d=D),
                            axis=mybir.AxisListType.X, op=OP.add)
                        ln2 = smallp.tile([P, K], f32, tag="ln2")
                        nc.scalar.activation(out=ln2[:], in_=ss2[:], func=AF.Ln, bias=eps_t[:])
                        rs2 = smallp.tile([P, K], f32, tag="rs2")
                        nc.scalar.activation(out=rs2[:], in_=ln2[:], func=AF.Exp, scale=-0.5)
                        if not last_it:
                            hnew = smallp.tile([P, F], b16, tag="hnew")
                            nc.vector.tensor_tensor(
                                out=hnew[:].rearrange("p (d k) -> p d k", d=D),
                                in0=attr[:].rearrange("p (d k) -> p d k", d=D),
                                in1=rs2[:].unsqueeze(1).to_broadcast([P, D, K]),
                                op=OP.mult)
                            st[j]["hd"] = hnew
                        else:
                            hout = smallp.tile([P, F], f32, tag="hout")
                            nc.vector.tensor_tensor(
                                out=hout[:].rearrange("p (d k) -> p d k", d=D),
                                in0=attr[:].rearrange("p (d k) -> p d k", d=D),
                                in1=rs2[:].unsqueeze(1).to_broadcast([P, D, K]),
                                op=OP.mult)
                            nc.sync.dma_start(out=out[j * P:(j + 1) * P, :], in_=hout[:])
    if not nc.is_finalized():
        nc.finalize()
    return nc


# --------------------------------------------------------------------------
# entry point
# --------------------------------------------------------------------------

def kernel(x, edge_index, weight, bias):
    x = np.asarray(x, dtype=np.float32)
    weight = np.asarray(weight, dtype=np.float32)
    bias = np.asarray(bias, dtype=np.float32)
    assert x.shape == (N_NODES, F) and edge_index.shape == (2, N_EDGES)

    metas, offs, node_order, cA, cB = _preprocess(edge_index)
    with_bias = bool(np.any(bias != 0))
    nc = _build(offs, cA, cB, with_bias, metas[0].shape[1])

    # device uses (d, k)-interleaved feature order: f' = d*K + k <-> f = k*D + d
    perm = np.array([k * D + d for d in range(D) for k in range(K)])
    xpad = np.zeros((NPAD, F), np.float32)
    xpad[:N_NODES] = x
    xt = np.ascontiguousarray(xpad.T)                        # [128, NPAD] f32
    wp = np.ascontiguousarray(weight[:, perm])
    const = np.zeros((P, 512), np.int16)
    const[:, 0:128] = np.tile(np.arange(P, dtype=bf16)[None, :], (P, 1)).view(np.int16)
    const[:, 128:256] = np.eye(P, dtype=bf16).view(np.int16)
    const[:, 256:512] = np.tile(bias[perm].astype(np.float32)[None, :], (P, 1)).view(np.int16)

    in_maps = [
        dict(xt=xt, w=wp, meta=metas[c], const=const) for c in range(NCORES)
    ]
    res = run_bass_kernel_spmd(nc, in_maps, core_ids=list(range(NCORES)), trace=TRACE)
    LAST_RESULTS["exec_time_ns"] = res.exec_time_ns
    LAST_RESULTS["trace"] = res.instructions_and_trace
    LAST_RESULTS["nc"] = nc
    LAST_RESULTS["in_maps"] = in_maps

    full = np.zeros((NPAD, F), np.float32)
    for c in range(NCORES):
        full[node_order[c][:, None], perm[None, :]] = res.results[c]["out"]
    return full[:N_NODES]


# revision 10
# speedup vs baseline: 2.1615x; 1.2180x over previous
"""Trainium2 Bass kernel for nn_DisenGCNLayer (disentangled GCN layer).

Strategy (8 NeuronCores, zero inter-core communication):
  - Destination nodes sharded across cores; each core owns 49 tiles of 128
    nodes and all edges pointing into them. Per tile, edges live in 128-wide
    chunks (edge-major SBUF layout).
  - h_src gathered once per tile via dma_gather (int16 indices, table split at
    row 32768 into A/B halves).
  - Per-iteration h_dst expansion is a PE matmul against a transposed 0/1
    onehotT (node -> edge slot) built on-device, NOT a DMA gather: the Q7
    descriptor generation for per-edge gathers (~9.4ns/row) was the baseline
    bottleneck. Expanded rows land in PSUM f32 and are copied to SBUF bf16 by
    the scalar engine so the DVE product runs in 2x mode.
  - Segment softmax needs no segment-max (scores are dots of unit vectors,
    |s| <= 1); denominator is factored out of the weighted aggregation, so
    denominator + aggregation reduce to ONE PSUM-accumulated matmul per chunk
    against the edge-major onehot.
  - All activation funcs (leaky_relu, exp, ln, square, copy) live in ONE act
    table set (natural_log_exp_and_others): rsqrt is computed as
    exp(-0.5*ln(x)) to avoid Sqrt (different table -> 1.3us reload per swap).
"""

import heapq

import numpy as np
import ml_dtypes

import concourse.bass as bass
import concourse.bacc as bacc
import concourse.mybir as mybir
import concourse.tile as tile
from concourse.bass_utils import run_bass_kernel_spmd

bf16 = ml_dtypes.bfloat16

# problem spec (hardcoded)
N_NODES = 50000
N_EDGES = 800000
F = 128
K = 8
D = 16
ITERS = 4

NCORES = 8
P = 128
TILES = 392                 # total node tiles
TPC = TILES // NCORES       # 49 tiles per core
NPC = TPC * P               # 6272 nodes per core
NPAD = TILES * P            # 50176
SPLIT = 32768               # int16 gather index limit split

PGRP = 4                    # phase-0 node-chunk group size
CB = 8                      # expansion matmul PSUM batch (chunks)

TRACE = False               # test.py sets kernel.TRACE = True for profiling
DEBUG_STAGE = 99            # bisect: 1=phase0 only, 99=full
LAST_RESULTS = {}           # exec info stash for test.py


# --------------------------------------------------------------------------
# host-side preprocessing
# --------------------------------------------------------------------------

def _preprocess(edge_index):
    row = np.asarray(edge_index[0], dtype=np.int64).astype(np.int32)
    col = np.asarray(edge_index[1], dtype=np.int64).astype(np.int32)

    degA = np.bincount(row[col < SPLIT], minlength=NPAD).astype(np.int64)
    degB = np.bincount(row[col >= SPLIT], minlength=NPAD).astype(np.int64)
    deg = degA + degB

    # --- bin-pack nodes into 392 tiles of exactly 128 nodes, balancing edges
    order = np.argsort(-deg, kind="stable")
    heap = [(0, t) for t in range(TILES)]
    heapq.heapify(heap)
    tile_nodes = [[] for _ in range(TILES)]
    tile_load = np.zeros(TILES, np.int64)
    for n in order:
        while True:
            load, t = heapq.heappop(heap)
            if len(tile_nodes[t]) < P:
                break
        tile_nodes[t].append(n)
        tile_load[t] = load + deg[n]
        if len(tile_nodes[t]) < P:
            heapq.heappush(heap, (tile_load[t], t))
    tileA = np.array([degA[tile_nodes[t]].sum() for t in range(TILES)])
    tileB = np.array([degB[tile_nodes[t]].sum() for t in range(TILES)])

    # --- assign tiles to cores: snake by total load, then sort each core's
    # tiles by nA desc so slot j is similar across cores (slot chunk counts
    # are maxed across cores and must be program-uniform).
    t_order = np.argsort(-(tileA + tileB), kind="stable")
    core_tiles = [[] for _ in range(NCORES)]
    for i, t in enumerate(t_order):
        c = i % (2 * NCORES)
        c = c if c < NCORES else 2 * NCORES - 1 - c
        core_tiles[c].append(t)
    for c in range(NCORES):
        core_tiles[c].sort(key=lambda t: -tileA[t])

    cA = np.zeros(TPC, np.int64)
    cB = np.zeros(TPC, np.int64)

    # --- per-node tile assignment + local slot
    node_tile = np.empty(NPAD, np.int32)
    node_local = np.empty(NPAD, np.int32)
    for t in range(TILES):
        ids = np.sort(np.array(tile_nodes[t], np.int64))
        tile_nodes[t] = ids
        node_tile[ids] = t
        node_local[ids] = np.arange(P, dtype=np.int32)

    # --- group edges by (tile, A/B)
    isB = (col >= SPLIT).astype(np.int64)
    ekey = node_tile[row].astype(np.int64) * 2 + isB
    eorder = np.argsort(ekey, kind="stable")
    ekey_s = ekey[eorder]
    col_s = col[eorder]
    seg_s = node_local[row[eorder]]
    # start offset of each (tile, side) group
    bounds = np.searchsorted(ekey_s, np.arange(2 * TILES + 1))

    def side_arrays(t, side, S_slots):
        """Slot assignment for one (tile, side): edges in seg order, slot s at
        (partition s%128, chunk s//128). Padded slots: col 0, seg -1."""
        a0, a1 = bounds[2 * t + side], bounds[2 * t + side + 1]
        segs = seg_s[a0:a1]
        cols = col_s[a0:a1] - (SPLIT if side else 0)
        o2 = np.argsort(segs, kind="stable")
        segs = segs[o2].astype(np.int64)
        cols = cols[o2].astype(np.int64)
        n = len(segs)
        assert n <= S_slots, (n, S_slots)
        slot_col = np.zeros(S_slots, np.int16)
        slot_seg = np.full(S_slots, -1.0, np.float32)
        slot_col[:n] = cols
        slot_seg[:n] = segs
        return slot_col, slot_seg

    def nedges(t, side):
        a0, a1 = bounds[2 * t + side], bounds[2 * t + side + 1]
        return int(a1 - a0)

    # chunk counts per tile slot, maxed across cores (program-uniform)
    for j in range(TPC):
        ca = cb = 0
        for c in range(NCORES):
            t = core_tiles[c][j]
            ca = max(ca, -(-nedges(t, 0) // P))
            cb = max(cb, -(-nedges(t, 1) // P))
        cA[j] = max(ca, 1)
        cB[j] = max(cb, 1)

    def wrap16(idx, nslots):
        a = np.zeros(nslots, np.int16)
        a[: len(idx)] = idx
        return np.tile(np.ascontiguousarray(a.reshape(-1, 16).T), (8, 1))

    def wrap128(vals, nslots):
        a = np.full(nslots, -1.0, np.float32)
        a[: len(vals)] = vals
        return np.ascontiguousarray(a.reshape(-1, P).T)

    # --- per-core meta tensor (identical layout across cores)
    metas = []
    tile_meta_off = []
    node_order = []  # per core: global ids in output-row order
    for c in range(NCORES):
        parts = []
        offs = []
        off = 0
        ids_order = []
        for j in range(TPC):
            t = core_tiles[c][j]
            ids = tile_nodes[t]
            ids_order.append(ids)
            SA, SB = int(cA[j]) * P, int(cB[j]) * P
            C = int(cA[j] + cB[j])
            colA, segA = side_arrays(t, 0, SA)
            colB, segB = side_arrays(t, 1, SB)
            ninit = ids.astype(np.int32)[:, None]
            idxA = wrap16(colA, SA)
            idxB = wrap16(colB, SB)
            seg = np.concatenate([wrap128(segA, SA), wrap128(segB, SB)], 1).astype(bf16)
            mt = np.concatenate(
                [ninit.view(np.int16).reshape(P, 2),
                 seg.view(np.int16),
                 idxA, idxB], 1)
            if mt.shape[1] % 2:
                mt = np.concatenate([mt, np.zeros((P, 1), np.int16)], 1)
            offs.append((off, C))
            off += mt.shape[1]
            parts.append(mt)
        metas.append(np.ascontiguousarray(np.concatenate(parts, 1)))
        tile_meta_off.append(offs)
        node_order.append(np.concatenate(ids_order))
    # layout identical across cores
    assert all(o == tile_meta_off[0] for o in tile_meta_off[1:])
    assert all(m.shape == metas[0].shape for m in metas)
    return metas, tile_meta_off[0], node_order, cA, cB


# --------------------------------------------------------------------------
# device program
# --------------------------------------------------------------------------

def _build(offs, cA, cB, with_bias, meta_w):
    f32, b16, i16, i32 = (mybir.dt.float32, mybir.dt.bfloat16,
                          mybir.dt.int16, mybir.dt.int32)
    nc = bacc.Bacc()
    xt_in = nc.declare_dram_parameter("xt", [P, NPAD], f32, isOutput=False)
    w_in = nc.declare_dram_parameter("w", [F, F], f32, isOutput=False)
    meta_in = nc.declare_dram_parameter("meta", [P, meta_w], i16, isOutput=False)
    const_in = nc.declare_dram_parameter("const", [P, 512], i16, isOutput=False)
    out = nc.declare_dram_parameter("out", [NPC, F], f32, isOutput=True)

    h16 = nc.dram_tensor("h16", [NPAD, F], b16)

    AF = mybir.ActivationFunctionType
    OP = mybir.AluOpType

    with tile.TileContext(nc) as tc:
        with (
            tc.tile_pool(name="const", bufs=1) as constp,
            tc.tile_pool(name="meta", bufs=4) as metap,
            tc.tile_pool(name="xin", bufs=2) as xinp,
            tc.tile_pool(name="ph0", bufs=2) as ph0p,
            tc.tile_pool(name="hl", bufs=8) as hlp,
            tc.tile_pool(name="big", bufs=7) as bigp,
            tc.tile_pool(name="hdp", bufs=4) as hdp,
            tc.tile_pool(name="small", bufs=8) as smallp,
            tc.tile_pool(name="pswork", bufs=2, space="PSUM") as pswork,
            tc.tile_pool(name="psda", bufs=2, space="PSUM") as psdap,
        ):
            # ---- constants
            ct = constp.tile([P, 512], i16)
            nc.sync.dma_start(out=ct[:], in_=const_in[:])
            iota_t = ct[:, 0:128].bitcast(b16)           # [128,128] row iota
            ident_t = ct[:, 128:256].bitcast(b16)        # [128,128] identity
            bias_t = ct[:, 256:512].bitcast(f32)         # [128,128] bias rows
            wt = constp.tile([P, F], f32)
            nc.sync.dma_start(out=wt[:], in_=w_in[:])
            eps_t = constp.tile([P, 1], f32)
            nc.vector.memset(eps_t[:], 1e-12)

            # ---- phase 0: h = normalize_k(leaky_relu(x @ W + b)) for all nodes
            NG = NPAD // (P * PGRP)                      # 98 groups
            SG = 7                                        # super-group: batch ln/exp
            for sg0 in range(0, NG, SG):
                sgn = min(SG, NG - sg0)
                hls = []
                ss = ph0p.tile([P, SG, PGRP, K], f32, tag="ss")
                for si in range(sgn):
                    gi = sg0 + si
                    xts = xinp.tile([P, PGRP * F], f32, tag="xts")
                    nc.sync.dma_start(out=xts[:], in_=xt_in[:, gi * PGRP * P:(gi + 1) * PGRP * P])
                    hl = hlp.tile([P, PGRP, F], f32, tag="hl")
                    for j in range(PGRP):
                        hp = pswork.tile([P, CB, F], f32, tag="pse")
                        nc.tensor.matmul(out=hp[:, 0, :], lhsT=xts[:, j * P:(j + 1) * P], rhs=wt[:], start=True, stop=True)
                        if with_bias:
                            hb = xinp.tile([P, F], f32, tag="hb")
                            nc.vector.tensor_tensor(out=hb[:], in0=hp[:, 0, :], in1=bias_t, op=OP.add)
                            nc.scalar.activation(out=hl[:, j, :], in_=hb[:], func=AF.Prelu, alpha=0.01)
                        else:
                            nc.scalar.activation(out=hl[:, j, :], in_=hp[:, 0, :], func=AF.Prelu, alpha=0.01)
                    sq = ph0p.tile([P, PGRP, F], f32, tag="sq")
                    nc.vector.tensor_tensor(out=sq[:], in0=hl[:], in1=hl[:], op=OP.mult)
                    nc.vector.tensor_reduce(
                        out=ss[:, si], in_=sq[:].rearrange("p g (d k) -> p g k d", d=D),
                        axis=mybir.AxisListType.X, op=OP.add)
                    hls.append(hl)
                lnv = ph0p.tile([P, SG, PGRP, K], f32, tag="lnv")
                nc.scalar.activation(out=lnv[:, 0:sgn], in_=ss[:, 0:sgn], func=AF.Ln, bias=eps_t[:])
                rs = ph0p.tile([P, SG, PGRP, K], f32, tag="rs")
                nc.scalar.activation(out=rs[:, 0:sgn], in_=lnv[:, 0:sgn], func=AF.Exp, scale=-0.5)
                for si in range(sgn):
                    gi = sg0 + si
                    hn16 = ph0p.tile([P, PGRP, F], b16, tag="hn16")
                    nc.vector.tensor_tensor(
                        out=hn16[:].rearrange("p g (d k) -> p g d k", d=D),
                        in0=hls[si][:].rearrange("p g (d k) -> p g d k", d=D),
                        in1=rs[:, si].unsqueeze(2).to_broadcast([P, PGRP, D, K]),
                        op=OP.mult)
                    r0 = gi * PGRP * P
                    nc.sync.dma_start(
                        out=h16[r0:r0 + PGRP * P, :].rearrange("(g p) f -> p g f", p=P),
                        in_=hn16[:])

            if DEBUG_STAGE == 1:
                zt = smallp.tile([P, F], f32, tag="zt")
                nc.vector.memset(zt[:], 0.0)
                for j in range(TPC):
                    nc.sync.dma_start(out=out[j * P:(j + 1) * P, :], in_=zt[:])

            # ---- iterations: tiles interleaved in groups to keep engines busy
            G = 7
            for grp0 in (range(0, TPC, G) if DEBUG_STAGE != 1 else []):
                grp = list(range(grp0, min(grp0 + G, TPC)))
                st = {}
                for j in grp:
                    off, C = offs[j]
                    SA, SB = int(cA[j]), int(cB[j])
                    W_t = 2 + C + 8 * (SA + SB) * 16 // 16
                    W_t = 2 + C + 8 * SA + 8 * SB
                    W_t += W_t % 2
                    mt = metap.tile([P, W_t], i16, tag="mt")
                    nc.sync.dma_start(out=mt[:], in_=meta_in[:, off:off + W_t])
                    o = 2
                    seg_t = mt[:, o:o + C].bitcast(b16); o += C
                    idxA_t = mt[:, o:o + 8 * SA]; o += 8 * SA
                    idxB_t = mt[:, o:o + 8 * SB]; o += 8 * SB
                    ninit_t = mt[:, 0:2].bitcast(i32)

                    # tile init: gather this tile's h_normed rows (bf16)
                    hnt = smallp.tile([P, F], b16, tag="hnt")
                    nc.gpsimd.indirect_dma_start(
                        out=hnt[:], out_offset=None, in_=h16[:],
                        in_offset=bass.IndirectOffsetOnAxis(ap=ninit_t[:, :1], axis=0))

                    # gather h_src for all edges (once per tile, reused 4 iters)
                    g = bigp.tile([P, C, F], b16, tag="g")
                    if SA:
                        nc.gpsimd.dma_gather(
                            out_ap=g[:, 0:SA, :], in_ap=h16[0:SPLIT, :], idxs_ap=idxA_t,
                            num_idxs=SA * P, num_idxs_reg=SA * P, elem_size=F, single_packet=False)
                    if SB:
                        nc.gpsimd.dma_gather(
                            out_ap=g[:, SA:C, :], in_ap=h16[SPLIT:NPAD, :], idxs_ap=idxB_t,
                            num_idxs=SB * P, num_idxs_reg=SB * P, elem_size=F, single_packet=False)

                    # edge-major onehot (for aggregation matmul)
                    onehot = bigp.tile([P, C, P], b16, tag="onehot")
                    nc.vector.tensor_tensor(
                        out=onehot[:],
                        in0=seg_t.unsqueeze(-1).to_broadcast([P, C, P]),
                        in1=iota_t.unsqueeze(1).to_broadcast([P, C, P]),
                        op=OP.is_equal)
                    # transposed onehot (for hd expansion matmul), via PE
                    onehotT = bigp.tile([P, C, P], b16, tag="onehotT")
                    for cb0 in range(0, C, CB):
                        nb = min(CB, C - cb0)
                        psTf = pswork.tile([P, CB, F], f32, tag="pse")
                        psT = psTf[:].bitcast(b16)
                        for c in range(nb):
                            nc.tensor.transpose(
                                out=psT[:, c, 0:P], in_=onehot[:, cb0 + c, :], identity=ident_t)
                        nc.scalar.activation(
                            out=onehotT[:, cb0:cb0 + nb, :], in_=psT[:, 0:nb, 0:P], func=AF.Copy)

                    st[j] = dict(C=C, g=g, onehot=onehot, onehotT=onehotT, hnt=hnt,
                                 hd=hnt)

                for it in range(ITERS):
                    last_it = it == ITERS - 1
                    # expansion + prod (per PSUM batch of CB chunks)
                    for j in grp:
                        C, g, onehotT, hd = (st[j][k] for k in ("C", "g", "onehotT", "hd"))
                        prod = hdp.tile([P, C, F], b16, tag="prod")
                        for cb0 in range(0, C, CB):
                            nb = min(CB, C - cb0)
                            pse = pswork.tile([P, CB, F], f32, tag="pse")
                            for c in range(nb):
                                nc.tensor.matmul(
                                    out=pse[:, c, :], lhsT=onehotT[:, cb0 + c, :], rhs=hd[:],
                                    start=True, stop=True)
                            hdx = hdp.tile([P, CB, F], b16, tag="hdx")
                            nc.scalar.activation(out=hdx[:, 0:nb, :], in_=pse[:, 0:nb, :], func=AF.Copy)
                            nc.vector.tensor_tensor(
                                out=prod[:, cb0:cb0 + nb, :], in0=g[:, cb0:cb0 + nb, :],
                                in1=hdx[:, 0:nb, :], op=OP.mult)
                        st[j]["prod"] = prod
                    # scores (reduce tree) + exp + weighted combo
                    for j in grp:
                        C, g, prod = st[j]["C"], st[j]["g"], st[j]["prod"]
                        pv = prod[:].rearrange("p c (d k) -> p c d k", d=D)
                        t1 = hdp.tile([P, C, 8, K], b16, tag="t1")
                        nc.vector.tensor_tensor(out=t1[:], in0=pv[:, :, 0:8, :], in1=pv[:, :, 8:16, :], op=OP.add)
                        t2 = hdp.tile([P, C, 4, K], b16, tag="t2")
                        nc.vector.tensor_tensor(out=t2[:], in0=t1[:, :, 0:4, :], in1=t1[:, :, 4:8, :], op=OP.add)
                        t3 = hdp.tile([P, C, 2, K], b16, tag="t3")
                        nc.vector.tensor_tensor(out=t3[:], in0=t2[:, :, 0:2, :], in1=t2[:, :, 2:4, :], op=OP.add)
                        scores = hdp.tile([P, C, K], f32, tag="scores")
                        nc.vector.tensor_tensor(
                            out=scores[:].unsqueeze(2), in0=t3[:, :, 0:1, :], in1=t3[:, :, 1:2, :], op=OP.add)
                        combo = hdp.tile([P, C, K + F], b16, tag="combo")
                        nc.scalar.activation(out=combo[:, :, 0:K], in_=scores[:], func=AF.Exp)
                        nc.vector.tensor_tensor(
                            out=combo[:, :, K:K + F].rearrange("p c (d k) -> p c d k", d=D),
                            in0=g[:].rearrange("p c (d k) -> p c d k", d=D),
                            in1=combo[:, :, 0:K].unsqueeze(2).to_broadcast([P, C, D, K]),
                            op=OP.mult)
                        st[j]["combo"] = combo
                    # aggregation matmul (PSUM-accumulated over chunks)
                    for j in grp:
                        C, onehot, combo = st[j]["C"], st[j]["onehot"], st[j]["combo"]
                        da = psdap.tile([P, K + F], f32, tag="da")
                        for c in range(C):
                            nc.tensor.matmul(out=da[:], lhsT=onehot[:, c, :], rhs=combo[:, c, :],
                                             start=(c == 0), stop=(c == C - 1))
                        st[j]["da"] = da
                    # epilogue: normalize (ln/exp batched per group), update hd
                    ss2g = smallp.tile([P, G, K], f32, tag="ss2g")
                    attrs = {}
                    for gj, j in enumerate(grp):
                        C, da, hnt = st[j]["C"], st[j]["da"], st[j]["hnt"]
                        deps = smallp.tile([P, K], f32, tag="deps")
                        nc.vector.tensor_scalar_add(deps[:], da[:, 0:K], 1e-6)
                        rden = smallp.tile([P, K], f32, tag="rden")
                        nc.vector.reciprocal(out=rden[:], in_=deps[:])
                        attr = smallp.tile([P, F], f32, tag="attr")
                        nc.vector.tensor_tensor(
                            out=attr[:].rearrange("p (d k) -> p d k", d=D),
                            in0=da[:, K:K + F].rearrange("p (d k) -> p d k", d=D),
                            in1=rden[:].unsqueeze(1).to_broadcast([P, D, K]),
                            op=OP.mult)
                        nc.vector.tensor_tensor(out=attr[:], in0=attr[:], in1=hnt[:], op=OP.add)
                        sq2 = smallp.tile([P, F], f32, tag="sq2")
                        nc.vector.tensor_tensor(out=sq2[:], in0=attr[:], in1=attr[:], op=OP.mult)
                        nc.vector.tensor_reduce(
                            out=ss2g[:, gj], in_=sq2[:].rearrange("p (d k) -> p k d", d=D),
                            axis=mybir.AxisListType.X, op=OP.add)
                        attrs[j] = attr
                    ng = len(grp)
                    ln2g = smallp.tile([P, G, K], f32, tag="ln2g")
                    nc.scalar.activation(out=ln2g[:, 0:ng], in_=ss2g[:, 0:ng], func=AF.Ln, bias=eps_t[:])
                    rs2g = smallp.tile([P, G, K], f32, tag="rs2g")
                    nc.scalar.activation(out=rs2g[:, 0:ng], in_=ln2g[:, 0:ng], func=AF.Exp, scale=-0.5)
                    for gj, j in enumerate(grp):
                        attr = attrs[j]
                        if not last_it:
                            hnew = smallp.tile([P, F], b16, tag="hnew")
                            nc.vector.tensor_tensor(
                                out=hnew[:].rearrange("p (d k) -> p d k", d=D),
                                in0=attr[:].rearrange("p (d k) -> p d k", d=D),
                                in1=rs2g[:, gj].unsqueeze(1).to_broadcast([P, D, K]),
                                op=OP.mult)
                            st[j]["hd"] = hnew
                        else:
                            hout = smallp.tile([P, F], f32, tag="hout")
                            nc.vector.tensor_tensor(
                                out=hout[:].rearrange("p (d k) -> p d k", d=D),
                                in0=attr[:].rearrange("p (d k) -> p d k", d=D),
                                in1=rs2g[:, gj].unsqueeze(1).to_broadcast([P, D, K]),
                                op=OP.mult)
                            nc.sync.dma_start(out=out[j * P:(j + 1) * P, :], in_=hout[:])
    if not nc.is_finalized():
        nc.finalize()
    return nc


# --------------------------------------------------------------------------
# entry point
# --------------------------------------------------------------------------

def kernel(x, edge_index, weight, bias):
    x = np.asarray(x, dtype=np.float32)
    weight = np.asarray(weight, dtype=np.float32)
    bias = np.asarray(bias, dtype=np.float32)
    assert x.shape == (N_NODES, F) and edge_index.shape == (2, N_EDGES)

    metas, offs, node_order, cA, cB = _preprocess(edge_index)
    with_bias = bool(np.any(bias != 0))
    nc = _build(offs, cA, cB, with_bias, metas[0].shape[1])

    # device uses (d, k)-interleaved feature order: f' = d*K + k <-> f = k*D + d
    perm = np.array([k * D + d for d in range(D) for k in range(K)])
    xpad = np.zeros((NPAD, F), np.float32)
    xpad[:N_NODES] = x
    xt = np.ascontiguousarray(xpad.T)                        # [128, NPAD] f32
    wp = np.ascontiguousarray(weight[:, perm])
    const = np.zeros((P, 512), np.int16)
    const[:, 0:128] = np.tile(np.arange(P, dtype=bf16)[None, :], (P, 1)).view(np.int16)
    const[:, 128:256] = np.eye(P, dtype=bf16).view(np.int16)
    const[:, 256:512] = np.tile(bias[perm].astype(np.float32)[None, :], (P, 1)).view(np.int16)

    in_maps = [
        dict(xt=xt, w=wp, meta=metas[c], const=const) for c in range(NCORES)
    ]
    res = run_bass_kernel_spmd(nc, in_maps, core_ids=list(range(NCORES)), trace=TRACE)
    LAST_RESULTS["exec_time_ns"] = res.exec_time_ns
    LAST_RESULTS["trace"] = res.instructions_and_trace
    LAST_RESULTS["nc"] = nc
    LAST_RESULTS["in_maps"] = in_maps

    full = np.zeros((NPAD, F), np.float32)
    for c in range(NCORES):
        full[node_order[c][:, None], perm[None, :]] = res.results[c]["out"]
    return full[:N_NODES]


# revision 11
# speedup vs baseline: 2.6422x; 1.2224x over previous
"""Trainium2 Bass kernel for nn_DisenGCNLayer (disentangled GCN layer).

Strategy (8 NeuronCores, zero inter-core communication):
  - Destination nodes sharded across cores; each core owns 49 tiles of 128
    nodes and all edges pointing into them. Per tile, edges live in 128-wide
    chunks (edge-major SBUF layout).
  - h_src gathered once per tile via dma_gather (int16 indices, table split at
    row 32768 into A/B halves).
  - Per-iteration h_dst expansion is a PE matmul against a transposed 0/1
    onehotT (node -> edge slot) built on-device, NOT a DMA gather: the Q7
    descriptor generation for per-edge gathers (~9.4ns/row) was the baseline
    bottleneck. Expanded rows land in PSUM f32 and are copied to SBUF bf16 by
    the scalar engine so the DVE product runs in 2x mode.
  - Segment softmax needs no segment-max (scores are dots of unit vectors,
    |s| <= 1); denominator is factored out of the weighted aggregation, so
    denominator + aggregation reduce to ONE PSUM-accumulated matmul per chunk
    against the edge-major onehot.
  - All activation funcs (leaky_relu, exp, ln, square, copy) live in ONE act
    table set (natural_log_exp_and_others): rsqrt is computed as
    exp(-0.5*ln(x)) to avoid Sqrt (different table -> 1.3us reload per swap).
"""

import heapq

import numpy as np
import ml_dtypes

import concourse.bass as bass
import concourse.bacc as bacc
import concourse.mybir as mybir
import concourse.tile as tile
from concourse.bass_utils import run_bass_kernel_spmd

bf16 = ml_dtypes.bfloat16

# problem spec (hardcoded)
N_NODES = 50000
N_EDGES = 800000
F = 128
K = 8
D = 16
ITERS = 4

NCORES = 8
P = 128
TILES = 392                 # total node tiles
TPC = TILES // NCORES       # 49 tiles per core
NPC = TPC * P               # 6272 nodes per core
NPAD = TILES * P            # 50176
SPLIT = 32768               # int16 gather index limit split

PGRP = 4                    # phase-0 node-chunk group size
CB = 8                      # expansion matmul PSUM batch (chunks)

TRACE = False               # test.py sets kernel.TRACE = True for profiling
DEBUG_STAGE = 99            # bisect: 1=phase0 only, 99=full
LAST_RESULTS = {}           # exec info stash for test.py


# --------------------------------------------------------------------------
# host-side preprocessing
# --------------------------------------------------------------------------

def _preprocess(edge_index):
    row = np.asarray(edge_index[0], dtype=np.int64).astype(np.int32)
    col = np.asarray(edge_index[1], dtype=np.int64).astype(np.int32)

    degA = np.bincount(row[col < SPLIT], minlength=NPAD).astype(np.int64)
    degB = np.bincount(row[col >= SPLIT], minlength=NPAD).astype(np.int64)
    deg = degA + degB

    # --- bin-pack nodes into 392 tiles of exactly 128 nodes, balancing edges
    order = np.argsort(-deg, kind="stable")
    heap = [(0, t) for t in range(TILES)]
    heapq.heapify(heap)
    tile_nodes = [[] for _ in range(TILES)]
    tile_load = np.zeros(TILES, np.int64)
    for n in order:
        while True:
            load, t = heapq.heappop(heap)
            if len(tile_nodes[t]) < P:
                break
        tile_nodes[t].append(n)
        tile_load[t] = load + deg[n]
        if len(tile_nodes[t]) < P:
            heapq.heappush(heap, (tile_load[t], t))
    tileA = np.array([degA[tile_nodes[t]].sum() for t in range(TILES)])
    tileB = np.array([degB[tile_nodes[t]].sum() for t in range(TILES)])

    # --- assign tiles to cores: snake by total load, then sort each core's
    # tiles by nA desc so slot j is similar across cores (slot chunk counts
    # are maxed across cores and must be program-uniform).
    t_order = np.argsort(-(tileA + tileB), kind="stable")
    core_tiles = [[] for _ in range(NCORES)]
    for i, t in enumerate(t_order):
        c = i % (2 * NCORES)
        c = c if c < NCORES else 2 * NCORES - 1 - c
        core_tiles[c].append(t)
    for c in range(NCORES):
        core_tiles[c].sort(key=lambda t: -tileA[t])

    cA = np.zeros(TPC, np.int64)
    cB = np.zeros(TPC, np.int64)

    # --- per-node tile assignment + local slot
    node_tile = np.empty(NPAD, np.int32)
    node_local = np.empty(NPAD, np.int32)
    for t in range(TILES):
        ids = np.sort(np.array(tile_nodes[t], np.int64))
        tile_nodes[t] = ids
        node_tile[ids] = t
        node_local[ids] = np.arange(P, dtype=np.int32)

    # --- group edges by (tile, A/B)
    isB = (col >= SPLIT).astype(np.int64)
    ekey = node_tile[row].astype(np.int64) * 2 + isB
    eorder = np.argsort(ekey, kind="stable")
    ekey_s = ekey[eorder]
    col_s = col[eorder]
    seg_s = node_local[row[eorder]]
    # start offset of each (tile, side) group
    bounds = np.searchsorted(ekey_s, np.arange(2 * TILES + 1))

    def side_arrays(t, side, S_slots):
        """Slot assignment for one (tile, side): edges in seg order, slot s at
        (partition s%128, chunk s//128). Padded slots: col 0, seg -1."""
        a0, a1 = bounds[2 * t + side], bounds[2 * t + side + 1]
        segs = seg_s[a0:a1]
        cols = col_s[a0:a1] - (SPLIT if side else 0)
        o2 = np.argsort(segs, kind="stable")
        segs = segs[o2].astype(np.int64)
        cols = cols[o2].astype(np.int64)
        n = len(segs)
        assert n <= S_slots, (n, S_slots)
        slot_col = np.zeros(S_slots, np.int16)
        slot_seg = np.full(S_slots, -1.0, np.float32)
        slot_col[:n] = cols
        slot_seg[:n] = segs
        return slot_col, slot_seg

    def nedges(t, side):
        a0, a1 = bounds[2 * t + side], bounds[2 * t + side + 1]
        return int(a1 - a0)

    # chunk counts per tile slot, maxed across cores (program-uniform)
    for j in range(TPC):
        ca = cb = 0
        for c in range(NCORES):
            t = core_tiles[c][j]
            ca = max(ca, -(-nedges(t, 0) // P))
            cb = max(cb, -(-nedges(t, 1) // P))
        cA[j] = max(ca, 1)
        cB[j] = max(cb, 1)

    def wrap16(idx, nslots):
        a = np.zeros(nslots, np.int16)
        a[: len(idx)] = idx
        return np.tile(np.ascontiguousarray(a.reshape(-1, 16).T), (8, 1))

    def wrap128(vals, nslots):
        a = np.full(nslots, -1.0, np.float32)
        a[: len(vals)] = vals
        return np.ascontiguousarray(a.reshape(-1, P).T)

    # --- per-core meta tensor (identical layout across cores)
    metas = []
    tile_meta_off = []
    node_order = []  # per core: global ids in output-row order
    for c in range(NCORES):
        parts = []
        offs = []
        off = 0
        ids_order = []
        for j in range(TPC):
            t = core_tiles[c][j]
            ids = tile_nodes[t]
            ids_order.append(ids)
            SA, SB = int(cA[j]) * P, int(cB[j]) * P
            C = int(cA[j] + cB[j])
            colA, segA = side_arrays(t, 0, SA)
            colB, segB = side_arrays(t, 1, SB)
            ninit = ids.astype(np.int32)[:, None]
            idxA = wrap16(colA, SA)
            idxB = wrap16(colB, SB)
            seg = np.concatenate([wrap128(segA, SA), wrap128(segB, SB)], 1).astype(bf16)
            mt = np.concatenate(
                [ninit.view(np.int16).reshape(P, 2),
                 seg.view(np.int16),
                 idxA, idxB], 1)
            if mt.shape[1] % 2:
                mt = np.concatenate([mt, np.zeros((P, 1), np.int16)], 1)
            offs.append((off, C))
            off += mt.shape[1]
            parts.append(mt)
        metas.append(np.ascontiguousarray(np.concatenate(parts, 1)))
        tile_meta_off.append(offs)
        node_order.append(np.concatenate(ids_order))
    # layout identical across cores
    assert all(o == tile_meta_off[0] for o in tile_meta_off[1:])
    assert all(m.shape == metas[0].shape for m in metas)
    return metas, tile_meta_off[0], node_order, cA, cB


# --------------------------------------------------------------------------
# device program
# --------------------------------------------------------------------------

def _build(offs, cA, cB, with_bias, meta_w):
    f32, b16, i16, i32 = (mybir.dt.float32, mybir.dt.bfloat16,
                          mybir.dt.int16, mybir.dt.int32)
    nc = bacc.Bacc()
    xt_in = nc.declare_dram_parameter("xt", [P, NPAD], f32, isOutput=False)
    w_in = nc.declare_dram_parameter("w", [F, F], f32, isOutput=False)
    meta_in = nc.declare_dram_parameter("meta", [P, meta_w], i16, isOutput=False)
    const_in = nc.declare_dram_parameter("const", [P, 512], i16, isOutput=False)
    out = nc.declare_dram_parameter("out", [NPC, F], f32, isOutput=True)

    h16 = nc.dram_tensor("h16", [NPAD, F], b16)

    AF = mybir.ActivationFunctionType
    OP = mybir.AluOpType

    with tile.TileContext(nc) as tc:
        with (
            tc.tile_pool(name="const", bufs=1) as constp,
            tc.tile_pool(name="meta", bufs=4) as metap,
            tc.tile_pool(name="xin", bufs=2) as xinp,
            tc.tile_pool(name="ph0", bufs=2) as ph0p,
            tc.tile_pool(name="hl", bufs=8) as hlp,
            tc.tile_pool(name="gsrc", bufs=10) as gsrcp,
            tc.tile_pool(name="oh", bufs=7) as ohp,
            tc.tile_pool(name="hdp", bufs=3) as hdp,
            tc.tile_pool(name="small", bufs=8) as smallp,
            tc.tile_pool(name="pswork", bufs=2, space="PSUM") as pswork,
            tc.tile_pool(name="psda", bufs=2, space="PSUM") as psdap,
        ):
            # ---- constants
            ct = constp.tile([P, 512], i16)
            nc.sync.dma_start(out=ct[:], in_=const_in[:])
            iota_t = ct[:, 0:128].bitcast(b16)           # [128,128] row iota
            ident_t = ct[:, 128:256].bitcast(b16)        # [128,128] identity
            bias_t = ct[:, 256:512].bitcast(f32)         # [128,128] bias rows
            wt = constp.tile([P, F], f32)
            nc.sync.dma_start(out=wt[:], in_=w_in[:])
            eps_t = constp.tile([P, 1], f32)
            nc.vector.memset(eps_t[:], 1e-12)

            # ---- phase 0: h = normalize_k(leaky_relu(x @ W + b)) for all nodes
            NG = NPAD // (P * PGRP)                      # 98 groups
            SG = 7                                        # super-group: batch ln/exp
            for sg0 in range(0, NG, SG):
                sgn = min(SG, NG - sg0)
                hls = []
                ss = ph0p.tile([P, SG, PGRP, K], f32, tag="ss")
                for si in range(sgn):
                    gi = sg0 + si
                    xts = xinp.tile([P, PGRP * F], f32, tag="xts")
                    nc.sync.dma_start(out=xts[:], in_=xt_in[:, gi * PGRP * P:(gi + 1) * PGRP * P])
                    hl = hlp.tile([P, PGRP, F], f32, tag="hl")
                    for j in range(PGRP):
                        hp = pswork.tile([P, CB, F], f32, tag="pse")
                        nc.tensor.matmul(out=hp[:, 0, :], lhsT=xts[:, j * P:(j + 1) * P], rhs=wt[:], start=True, stop=True)
                        if with_bias:
                            hb = xinp.tile([P, F], f32, tag="hb")
                            nc.vector.tensor_tensor(out=hb[:], in0=hp[:, 0, :], in1=bias_t, op=OP.add)
                            nc.scalar.activation(out=hl[:, j, :], in_=hb[:], func=AF.Prelu, alpha=0.01)
                        else:
                            nc.scalar.activation(out=hl[:, j, :], in_=hp[:, 0, :], func=AF.Prelu, alpha=0.01)
                    sq = ph0p.tile([P, PGRP, F], f32, tag="sq")
                    nc.vector.tensor_tensor(out=sq[:], in0=hl[:], in1=hl[:], op=OP.mult)
                    nc.vector.tensor_reduce(
                        out=ss[:, si], in_=sq[:].rearrange("p g (d k) -> p g k d", d=D),
                        axis=mybir.AxisListType.X, op=OP.add)
                    hls.append(hl)
                lnv = ph0p.tile([P, SG, PGRP, K], f32, tag="lnv")
                nc.scalar.activation(out=lnv[:, 0:sgn], in_=ss[:, 0:sgn], func=AF.Ln, bias=eps_t[:])
                rs = ph0p.tile([P, SG, PGRP, K], f32, tag="rs")
                nc.scalar.activation(out=rs[:, 0:sgn], in_=lnv[:, 0:sgn], func=AF.Exp, scale=-0.5)
                for si in range(sgn):
                    gi = sg0 + si
                    hn16 = ph0p.tile([P, PGRP, F], b16, tag="hn16")
                    nc.vector.tensor_tensor(
                        out=hn16[:].rearrange("p g (d k) -> p g d k", d=D),
                        in0=hls[si][:].rearrange("p g (d k) -> p g d k", d=D),
                        in1=rs[:, si].unsqueeze(2).to_broadcast([P, PGRP, D, K]),
                        op=OP.mult)
                    r0 = gi * PGRP * P
                    nc.sync.dma_start(
                        out=h16[r0:r0 + PGRP * P, :].rearrange("(g p) f -> p g f", p=P),
                        in_=hn16[:])

            if DEBUG_STAGE == 1:
                zt = smallp.tile([P, F], f32, tag="zt")
                nc.vector.memset(zt[:], 0.0)
                for j in range(TPC):
                    nc.sync.dma_start(out=out[j * P:(j + 1) * P, :], in_=zt[:])

            # ---- iterations: tiles interleaved in groups to keep engines busy
            G = 5
            for grp0 in (range(0, TPC, G) if DEBUG_STAGE != 1 else []):
                grp = list(range(grp0, min(grp0 + G, TPC)))
                st = {}
                for j in grp:
                    off, C = offs[j]
                    SA, SB = int(cA[j]), int(cB[j])
                    W_t = 2 + C + 8 * (SA + SB) * 16 // 16
                    W_t = 2 + C + 8 * SA + 8 * SB
                    W_t += W_t % 2
                    mt = metap.tile([P, W_t], i16, tag="mt")
                    nc.sync.dma_start(out=mt[:], in_=meta_in[:, off:off + W_t])
                    o = 2
                    seg_t = mt[:, o:o + C].bitcast(b16); o += C
                    idxA_t = mt[:, o:o + 8 * SA]; o += 8 * SA
                    idxB_t = mt[:, o:o + 8 * SB]; o += 8 * SB
                    ninit_t = mt[:, 0:2].bitcast(i32)

                    # tile init: gather this tile's h_normed rows (bf16)
                    hnt = smallp.tile([P, F], b16, tag="hnt")
                    nc.gpsimd.indirect_dma_start(
                        out=hnt[:], out_offset=None, in_=h16[:],
                        in_offset=bass.IndirectOffsetOnAxis(ap=ninit_t[:, :1], axis=0))

                    # gather h_src for all edges (once per tile, reused 4 iters)
                    g = gsrcp.tile([P, C, F], b16, tag="g")
                    if SA:
                        nc.gpsimd.dma_gather(
                            out_ap=g[:, 0:SA, :], in_ap=h16[0:SPLIT, :], idxs_ap=idxA_t,
                            num_idxs=SA * P, num_idxs_reg=SA * P, elem_size=F, single_packet=False)
                    if SB:
                        nc.gpsimd.dma_gather(
                            out_ap=g[:, SA:C, :], in_ap=h16[SPLIT:NPAD, :], idxs_ap=idxB_t,
                            num_idxs=SB * P, num_idxs_reg=SB * P, elem_size=F, single_packet=False)

                    # edge-major onehot (for aggregation matmul)
                    onehot = ohp.tile([P, C, P], b16, tag="onehot")
                    nc.vector.tensor_tensor(
                        out=onehot[:],
                        in0=seg_t.unsqueeze(-1).to_broadcast([P, C, P]),
                        in1=iota_t.unsqueeze(1).to_broadcast([P, C, P]),
                        op=OP.is_equal)
                    # transposed onehot (for hd expansion matmul), via PE
                    onehotT = ohp.tile([P, C, P], b16, tag="onehotT")
                    for cb0 in range(0, C, CB):
                        nb = min(CB, C - cb0)
                        psTf = pswork.tile([P, CB, F], f32, tag="pse")
                        psT = psTf[:].bitcast(b16)
                        for c in range(nb):
                            nc.tensor.transpose(
                                out=psT[:, c, 0:P], in_=onehot[:, cb0 + c, :], identity=ident_t)
                        nc.scalar.activation(
                            out=onehotT[:, cb0:cb0 + nb, :], in_=psT[:, 0:nb, 0:P], func=AF.Copy)

                    st[j] = dict(C=C, g=g, onehot=onehot, onehotT=onehotT, hnt=hnt,
                                 hd=hnt)

                for it in range(ITERS):
                    last_it = it == ITERS - 1
                    # expansion + prod (per PSUM batch of CB chunks)
                    for j in grp:
                        C, g, onehotT, hd = (st[j][k] for k in ("C", "g", "onehotT", "hd"))
                        prod = hdp.tile([P, C, F], b16, tag="prod")
                        for cb0 in range(0, C, CB):
                            nb = min(CB, C - cb0)
                            pse = pswork.tile([P, CB, F], f32, tag="pse")
                            for c in range(nb):
                                nc.tensor.matmul(
                                    out=pse[:, c, :], lhsT=onehotT[:, cb0 + c, :], rhs=hd[:],
                                    start=True, stop=True)
                            hdx = hdp.tile([P, CB, F], b16, tag="hdx")
                            nc.scalar.activation(out=hdx[:, 0:nb, :], in_=pse[:, 0:nb, :], func=AF.Copy)
                            nc.vector.tensor_tensor(
                                out=prod[:, cb0:cb0 + nb, :], in0=g[:, cb0:cb0 + nb, :],
                                in1=hdx[:, 0:nb, :], op=OP.mult)
                        st[j]["prod"] = prod
                    # scores (reduce tree) + exp + weighted combo
                    for j in grp:
                        C, g, prod = st[j]["C"], st[j]["g"], st[j]["prod"]
                        pv = prod[:].rearrange("p c (d k) -> p c d k", d=D)
                        t1 = hdp.tile([P, C, 8, K], b16, tag="t1")
                        nc.vector.tensor_tensor(out=t1[:], in0=pv[:, :, 0:8, :], in1=pv[:, :, 8:16, :], op=OP.add)
                        t2 = hdp.tile([P, C, 4, K], b16, tag="t2")
                        nc.vector.tensor_tensor(out=t2[:], in0=t1[:, :, 0:4, :], in1=t1[:, :, 4:8, :], op=OP.add)
                        t3 = hdp.tile([P, C, 2, K], b16, tag="t3")
                        nc.vector.tensor_tensor(out=t3[:], in0=t2[:, :, 0:2, :], in1=t2[:, :, 2:4, :], op=OP.add)
                        scores = hdp.tile([P, C, K], f32, tag="scores")
                        nc.vector.tensor_tensor(
                            out=scores[:].unsqueeze(2), in0=t3[:, :, 0:1, :], in1=t3[:, :, 1:2, :], op=OP.add)
                        combo = hdp.tile([P, C, K + F], b16, tag="combo")
                        nc.scalar.activation(out=combo[:, :, 0:K], in_=scores[:], func=AF.Exp)
                        nc.vector.tensor_tensor(
                            out=combo[:, :, K:K + F].rearrange("p c (d k) -> p c d k", d=D),
                            in0=g[:].rearrange("p c (d k) -> p c d k", d=D),
                            in1=combo[:, :, 0:K].unsqueeze(2).to_broadcast([P, C, D, K]),
                            op=OP.mult)
                        st[j]["combo"] = combo
                    # aggregation matmul (PSUM-accumulated over chunks)
                    for j in grp:
                        C, onehot, combo = st[j]["C"], st[j]["onehot"], st[j]["combo"]
                        da = psdap.tile([P, K + F], f32, tag="da")
                        for c in range(C):
                            nc.tensor.matmul(out=da[:], lhsT=onehot[:, c, :], rhs=combo[:, c, :],
                                             start=(c == 0), stop=(c == C - 1))
                        st[j]["da"] = da
                    # epilogue: normalize (ln/exp batched per group), update hd
                    ss2g = smallp.tile([P, G, K], f32, tag="ss2g")
                    attrs = {}
                    for gj, j in enumerate(grp):
                        C, da, hnt = st[j]["C"], st[j]["da"], st[j]["hnt"]
                        deps = smallp.tile([P, K], f32, tag="deps")
                        nc.vector.tensor_scalar_add(deps[:], da[:, 0:K], 1e-6)
                        rden = smallp.tile([P, K], f32, tag="rden")
                        nc.vector.reciprocal(out=rden[:], in_=deps[:])
                        attr = smallp.tile([P, F], f32, tag="attr")
                        nc.vector.tensor_tensor(
                            out=attr[:].rearrange("p (d k) -> p d k", d=D),
                            in0=da[:, K:K + F].rearrange("p (d k) -> p d k", d=D),
                            in1=rden[:].unsqueeze(1).to_broadcast([P, D, K]),
                            op=OP.mult)
                        nc.vector.tensor_tensor(out=attr[:], in0=attr[:], in1=hnt[:], op=OP.add)
                        sq2 = smallp.tile([P, F], f32, tag="sq2")
                        nc.vector.tensor_tensor(out=sq2[:], in0=attr[:], in1=attr[:], op=OP.mult)
                        nc.vector.tensor_reduce(
                            out=ss2g[:, gj], in_=sq2[:].rearrange("p (d k) -> p k d", d=D),
                            axis=mybir.AxisListType.X, op=OP.add)
                        attrs[j] = attr
                    ng = len(grp)
                    ln2g = smallp.tile([P, G, K], f32, tag="ln2g")
                    nc.scalar.activation(out=ln2g[:, 0:ng], in_=ss2g[:, 0:ng], func=AF.Ln, bias=eps_t[:])
                    rs2g = smallp.tile([P, G, K], f32, tag="rs2g")
                    nc.scalar.activation(out=rs2g[:, 0:ng], in_=ln2g[:, 0:ng], func=AF.Exp, scale=-0.5)
                    for gj, j in enumerate(grp):
                        attr = attrs[j]
                        if not last_it:
                            hnew = smallp.tile([P, F], b16, tag="hnew")
                            nc.vector.tensor_tensor(
                                out=hnew[:].rearrange("p (d k) -> p d k", d=D),
                                in0=attr[:].rearrange("p (d k) -> p d k", d=D),
                                in1=rs2g[:, gj].unsqueeze(1).to_broadcast([P, D, K]),
                                op=OP.mult)
                            st[j]["hd"] = hnew
                        else:
                            hout = smallp.tile([P, F], f32, tag="hout")
                            nc.vector.tensor_tensor(
                                out=hout[:].rearrange("p (d k) -> p d k", d=D),
                                in0=attr[:].rearrange("p (d k) -> p d k", d=D),
                                in1=rs2g[:, gj].unsqueeze(1).to_broadcast([P, D, K]),
                                op=OP.mult)
                            nc.sync.dma_start(out=out[j * P:(j + 1) * P, :], in_=hout[:])
    if not nc.is_finalized():
        nc.finalize()
    return nc


# --------------------------------------------------------------------------
# entry point
# --------------------------------------------------------------------------

def kernel(x, edge_index, weight, bias):
    x = np.asarray(x, dtype=np.float32)
    weight = np.asarray(weight, dtype=np.float32)
    bias = np.asarray(bias, dtype=np.float32)
    assert x.shape == (N_NODES, F) and edge_index.shape == (2, N_EDGES)

    metas, offs, node_order, cA, cB = _preprocess(edge_index)
    with_bias = bool(np.any(bias != 0))
    nc = _build(offs, cA, cB, with_bias, metas[0].shape[1])

    # device uses (d, k)-interleaved feature order: f' = d*K + k <-> f = k*D + d
    perm = np.array([k * D + d for d in range(D) for k in range(K)])
    xpad = np.zeros((NPAD, F), np.float32)
    xpad[:N_NODES] = x
    xt = np.ascontiguousarray(xpad.T)                        # [128, NPAD] f32
    wp = np.ascontiguousarray(weight[:, perm])
    const = np.zeros((P, 512), np.int16)
    const[:, 0:128] = np.tile(np.arange(P, dtype=bf16)[None, :], (P, 1)).view(np.int16)
    const[:, 128:256] = np.eye(P, dtype=bf16).view(np.int16)
    const[:, 256:512] = np.tile(bias[perm].astype(np.float32)[None, :], (P, 1)).view(np.int16)

    in_maps = [
        dict(xt=xt, w=wp, meta=metas[c], const=const) for c in range(NCORES)
    ]
    res = run_bass_kernel_spmd(nc, in_maps, core_ids=list(range(NCORES)), trace=TRACE)
    LAST_RESULTS["exec_time_ns"] = res.exec_time_ns
    LAST_RESULTS["trace"] = res.instructions_and_trace
    LAST_RESULTS["nc"] = nc
    LAST_RESULTS["in_maps"] = in_maps

    full = np.zeros((NPAD, F), np.float32)
    for c in range(NCORES):
        full[node_order[c][:, None], perm[None, :]] = res.results[c]["out"]
    return full[:N_NODES]


# revision 13
# speedup vs baseline: 2.6817x; 1.0149x over previous
"""Trainium2 Bass kernel for nn_DisenGCNLayer (disentangled GCN layer).

Strategy (8 NeuronCores, zero inter-core communication):
  - Destination nodes sharded across cores; each core owns 49 tiles of 128
    nodes and all edges pointing into them. Per tile, edges live in 128-wide
    chunks (edge-major SBUF layout).
  - h_src gathered once per tile via dma_gather (int16 indices, table split at
    row 32768 into A/B halves).
  - Per-iteration h_dst expansion is a PE matmul against a transposed 0/1
    onehotT (node -> edge slot) built on-device, NOT a DMA gather: the Q7
    descriptor generation for per-edge gathers (~9.4ns/row) was the baseline
    bottleneck. Expanded rows land in PSUM f32 and are copied to SBUF bf16 by
    the scalar engine so the DVE product runs in 2x mode.
  - Segment softmax needs no segment-max (scores are dots of unit vectors,
    |s| <= 1); denominator is factored out of the weighted aggregation, so
    denominator + aggregation reduce to ONE PSUM-accumulated matmul per chunk
    against the edge-major onehot.
  - All activation funcs (leaky_relu, exp, ln, square, copy) live in ONE act
    table set (natural_log_exp_and_others): rsqrt is computed as
    exp(-0.5*ln(x)) to avoid Sqrt (different table -> 1.3us reload per swap).
"""

import heapq

import numpy as np
import ml_dtypes

import concourse.bass as bass
import concourse.bacc as bacc
import concourse.mybir as mybir
import concourse.tile as tile
from concourse.bass_utils import run_bass_kernel_spmd

bf16 = ml_dtypes.bfloat16

# problem spec (hardcoded)
N_NODES = 50000
N_EDGES = 800000
F = 128
K = 8
D = 16
ITERS = 4

NCORES = 8
P = 128
TILES = 392                 # total node tiles
TPC = TILES // NCORES       # 49 tiles per core
NPC = TPC * P               # 6272 nodes per core
NPAD = TILES * P            # 50176
SPLIT = 32768               # int16 gather index limit split

PGRP = 4                    # phase-0 node-chunk group size
CB = 8                      # expansion matmul PSUM batch (chunks)

TRACE = False               # test.py sets kernel.TRACE = True for profiling
DEBUG_STAGE = 99            # bisect: 1=phase0 only, 99=full
LAST_RESULTS = {}           # exec info stash for test.py


# --------------------------------------------------------------------------
# host-side preprocessing
# --------------------------------------------------------------------------

def _preprocess(edge_index):
    row = np.asarray(edge_index[0], dtype=np.int64).astype(np.int32)
    col = np.asarray(edge_index[1], dtype=np.int64).astype(np.int32)

    degA = np.bincount(row[col < SPLIT], minlength=NPAD).astype(np.int64)
    degB = np.bincount(row[col >= SPLIT], minlength=NPAD).astype(np.int64)
    deg = degA + degB

    # --- bin-pack nodes into 392 tiles of exactly 128 nodes, balancing edges
    order = np.argsort(-deg, kind="stable")
    heap = [(0, t) for t in range(TILES)]
    heapq.heapify(heap)
    tile_nodes = [[] for _ in range(TILES)]
    tile_load = np.zeros(TILES, np.int64)
    for n in order:
        while True:
            load, t = heapq.heappop(heap)
            if len(tile_nodes[t]) < P:
                break
        tile_nodes[t].append(n)
        tile_load[t] = load + deg[n]
        if len(tile_nodes[t]) < P:
            heapq.heappush(heap, (tile_load[t], t))
    tileA = np.array([degA[tile_nodes[t]].sum() for t in range(TILES)])
    tileB = np.array([degB[tile_nodes[t]].sum() for t in range(TILES)])

    # --- assign tiles to cores: snake by total load, then sort each core's
    # tiles by nA desc so slot j is similar across cores (slot chunk counts
    # are maxed across cores and must be program-uniform).
    t_order = np.argsort(-(tileA + tileB), kind="stable")
    core_tiles = [[] for _ in range(NCORES)]
    for i, t in enumerate(t_order):
        c = i % (2 * NCORES)
        c = c if c < NCORES else 2 * NCORES - 1 - c
        core_tiles[c].append(t)
    for c in range(NCORES):
        core_tiles[c].sort(key=lambda t: -tileA[t])

    cA = np.zeros(TPC, np.int64)
    cB = np.zeros(TPC, np.int64)

    # --- per-node tile assignment + local slot
    node_tile = np.empty(NPAD, np.int32)
    node_local = np.empty(NPAD, np.int32)
    for t in range(TILES):
        ids = np.sort(np.array(tile_nodes[t], np.int64))
        tile_nodes[t] = ids
        node_tile[ids] = t
        node_local[ids] = np.arange(P, dtype=np.int32)

    # --- group edges by (tile, A/B)
    isB = (col >= SPLIT).astype(np.int64)
    ekey = node_tile[row].astype(np.int64) * 2 + isB
    eorder = np.argsort(ekey, kind="stable")
    ekey_s = ekey[eorder]
    col_s = col[eorder]
    seg_s = node_local[row[eorder]]
    # start offset of each (tile, side) group
    bounds = np.searchsorted(ekey_s, np.arange(2 * TILES + 1))

    def side_arrays(t, side, S_slots):
        """Slot assignment for one (tile, side): edges in seg order, slot s at
        (partition s%128, chunk s//128). Padded slots: col 0, seg -1."""
        a0, a1 = bounds[2 * t + side], bounds[2 * t + side + 1]
        segs = seg_s[a0:a1]
        cols = col_s[a0:a1] - (SPLIT if side else 0)
        o2 = np.argsort(segs, kind="stable")
        segs = segs[o2].astype(np.int64)
        cols = cols[o2].astype(np.int64)
        n = len(segs)
        assert n <= S_slots, (n, S_slots)
        slot_col = np.zeros(S_slots, np.int16)
        slot_seg = np.full(S_slots, -1.0, np.float32)
        slot_col[:n] = cols
        slot_seg[:n] = segs
        return slot_col, slot_seg

    def nedges(t, side):
        a0, a1 = bounds[2 * t + side], bounds[2 * t + side + 1]
        return int(a1 - a0)

    # chunk counts per tile slot, maxed across cores (program-uniform)
    for j in range(TPC):
        ca = cb = 0
        for c in range(NCORES):
            t = core_tiles[c][j]
            ca = max(ca, -(-nedges(t, 0) // P))
            cb = max(cb, -(-nedges(t, 1) // P))
        cA[j] = max(ca, 1)
        cB[j] = max(cb, 1)

    def wrap16(idx, nslots):
        a = np.zeros(nslots, np.int16)
        a[: len(idx)] = idx
        return np.tile(np.ascontiguousarray(a.reshape(-1, 16).T), (8, 1))

    def wrap128(vals, nslots):
        a = np.full(nslots, -1.0, np.float32)
        a[: len(vals)] = vals
        return np.ascontiguousarray(a.reshape(-1, P).T)

    # --- per-core meta tensor (identical layout across cores)
    metas = []
    tile_meta_off = []
    node_order = []  # per core: global ids in output-row order
    for c in range(NCORES):
        parts = []
        offs = []
        off = 0
        ids_order = []
        for j in range(TPC):
            t = core_tiles[c][j]
            ids = tile_nodes[t]
            ids_order.append(ids)
            SA, SB = int(cA[j]) * P, int(cB[j]) * P
            C = int(cA[j] + cB[j])
            colA, segA = side_arrays(t, 0, SA)
            colB, segB = side_arrays(t, 1, SB)
            ninit = ids.astype(np.int32)[:, None]
            idxA = wrap16(colA, SA)
            idxB = wrap16(colB, SB)
            seg = np.concatenate([wrap128(segA, SA), wrap128(segB, SB)], 1).astype(bf16)
            mt = np.concatenate(
                [ninit.view(np.int16).reshape(P, 2),
                 seg.view(np.int16),
                 idxA, idxB], 1)
            if mt.shape[1] % 2:
                mt = np.concatenate([mt, np.zeros((P, 1), np.int16)], 1)
            offs.append((off, C))
            off += mt.shape[1]
            parts.append(mt)
        metas.append(np.ascontiguousarray(np.concatenate(parts, 1)))
        tile_meta_off.append(offs)
        node_order.append(np.concatenate(ids_order))
    # layout identical across cores
    assert all(o == tile_meta_off[0] for o in tile_meta_off[1:])
    assert all(m.shape == metas[0].shape for m in metas)
    return metas, tile_meta_off[0], node_order, cA, cB


# --------------------------------------------------------------------------
# device program
# --------------------------------------------------------------------------

def _build(offs, cA, cB, with_bias, meta_w):
    f32, b16, i16, i32 = (mybir.dt.float32, mybir.dt.bfloat16,
                          mybir.dt.int16, mybir.dt.int32)
    nc = bacc.Bacc()
    xt_in = nc.declare_dram_parameter("xt", [P, NPAD], f32, isOutput=False)
    w_in = nc.declare_dram_parameter("w", [F, F], f32, isOutput=False)
    meta_in = nc.declare_dram_parameter("meta", [P, meta_w], i16, isOutput=False)
    const_in = nc.declare_dram_parameter("const", [P, 512], i16, isOutput=False)
    out = nc.declare_dram_parameter("out", [NPC, F], f32, isOutput=True)

    h16 = nc.dram_tensor("h16", [NPAD, F], b16)

    AF = mybir.ActivationFunctionType
    OP = mybir.AluOpType

    with tile.TileContext(nc) as tc:
        with (
            tc.tile_pool(name="const", bufs=1) as constp,
            tc.tile_pool(name="meta", bufs=4) as metap,
            tc.tile_pool(name="xin", bufs=2) as xinp,
            tc.tile_pool(name="ph0", bufs=2) as ph0p,
            tc.tile_pool(name="hl", bufs=8) as hlp,
            tc.tile_pool(name="gsrc", bufs=10) as gsrcp,
            tc.tile_pool(name="oh", bufs=7) as ohp,
            tc.tile_pool(name="hdp", bufs=3) as hdp,
            tc.tile_pool(name="small", bufs=8) as smallp,
            tc.tile_pool(name="pswork", bufs=2, space="PSUM") as pswork,
            tc.tile_pool(name="psda", bufs=2, space="PSUM") as psdap,
        ):
            # ---- constants
            ct = constp.tile([P, 512], i16)
            nc.sync.dma_start(out=ct[:], in_=const_in[:])
            iota_t = ct[:, 0:128].bitcast(b16)           # [128,128] row iota
            ident_t = ct[:, 128:256].bitcast(b16)        # [128,128] identity
            bias_t = ct[:, 256:512].bitcast(f32)         # [128,128] bias rows
            wt = constp.tile([P, F], f32)
            nc.sync.dma_start(out=wt[:], in_=w_in[:])
            eps_t = constp.tile([P, 1], f32)
            nc.vector.memset(eps_t[:], 1e-12)

            # ---- phase 0: h = normalize_k(leaky_relu(x @ W + b)) for all nodes
            NG = NPAD // (P * PGRP)                      # 98 groups
            SG = 7                                        # super-group: batch ln/exp
            for sg0 in range(0, NG, SG):
                sgn = min(SG, NG - sg0)
                hls = []
                ss = ph0p.tile([P, SG, PGRP, K], f32, tag="ss")
                for si in range(sgn):
                    gi = sg0 + si
                    xts = xinp.tile([P, PGRP * F], f32, tag="xts")
                    nc.sync.dma_start(out=xts[:], in_=xt_in[:, gi * PGRP * P:(gi + 1) * PGRP * P])
                    hl = hlp.tile([P, PGRP, F], f32, tag="hl")
                    hp = pswork.tile([P, CB, F], f32, tag="pse")
                    for j in range(PGRP):
                        nc.tensor.matmul(out=hp[:, j, :], lhsT=xts[:, j * P:(j + 1) * P], rhs=wt[:], start=True, stop=True)
                    if with_bias:
                        hb = xinp.tile([P, PGRP, F], f32, tag="hb")
                        nc.vector.tensor_tensor(
                            out=hb[:], in0=hp[:, 0:PGRP, :],
                            in1=bias_t.unsqueeze(1).to_broadcast([P, PGRP, F]), op=OP.add)
                        nc.scalar.activation(out=hl[:], in_=hb[:], func=AF.Prelu, alpha=0.01)
                    else:
                        nc.scalar.activation(out=hl[:], in_=hp[:, 0:PGRP, :], func=AF.Prelu, alpha=0.01)
                    sq = ph0p.tile([P, PGRP, F], f32, tag="sq")
                    nc.vector.tensor_tensor(out=sq[:], in0=hl[:], in1=hl[:], op=OP.mult)
                    nc.vector.tensor_reduce(
                        out=ss[:, si], in_=sq[:].rearrange("p g (d k) -> p g k d", d=D),
                        axis=mybir.AxisListType.X, op=OP.add)
                    hls.append(hl)
                lnv = ph0p.tile([P, SG, PGRP, K], f32, tag="lnv")
                nc.scalar.activation(out=lnv[:, 0:sgn], in_=ss[:, 0:sgn], func=AF.Ln, bias=eps_t[:])
                rs = ph0p.tile([P, SG, PGRP, K], f32, tag="rs")
                nc.scalar.activation(out=rs[:, 0:sgn], in_=lnv[:, 0:sgn], func=AF.Exp, scale=-0.5)
                for si in range(sgn):
                    gi = sg0 + si
                    hn16 = ph0p.tile([P, PGRP, F], b16, tag="hn16")
                    nc.vector.tensor_tensor(
                        out=hn16[:].rearrange("p g (d k) -> p g d k", d=D),
                        in0=hls[si][:].rearrange("p g (d k) -> p g d k", d=D),
                        in1=rs[:, si].unsqueeze(2).to_broadcast([P, PGRP, D, K]),
                        op=OP.mult)
                    r0 = gi * PGRP * P
                    nc.sync.dma_start(
                        out=h16[r0:r0 + PGRP * P, :].rearrange("(g p) f -> p g f", p=P),
                        in_=hn16[:])

            if DEBUG_STAGE == 1:
                zt = smallp.tile([P, F], f32, tag="zt")
                nc.vector.memset(zt[:], 0.0)
                for j in range(TPC):
                    nc.sync.dma_start(out=out[j * P:(j + 1) * P, :], in_=zt[:])

            # ---- iterations: tiles interleaved in groups to keep engines busy
            G = 5
            for grp0 in (range(0, TPC, G) if DEBUG_STAGE != 1 else []):
                grp = list(range(grp0, min(grp0 + G, TPC)))
                st = {}
                for j in grp:
                    off, C = offs[j]
                    SA, SB = int(cA[j]), int(cB[j])
                    W_t = 2 + C + 8 * (SA + SB) * 16 // 16
                    W_t = 2 + C + 8 * SA + 8 * SB
                    W_t += W_t % 2
                    mt = metap.tile([P, W_t], i16, tag="mt")
                    nc.sync.dma_start(out=mt[:], in_=meta_in[:, off:off + W_t])
                    o = 2
                    seg_t = mt[:, o:o + C].bitcast(b16); o += C
                    idxA_t = mt[:, o:o + 8 * SA]; o += 8 * SA
                    idxB_t = mt[:, o:o + 8 * SB]; o += 8 * SB
                    ninit_t = mt[:, 0:2].bitcast(i32)

                    # tile init: gather this tile's h_normed rows (bf16)
                    hnt = smallp.tile([P, F], b16, tag="hnt")
                    nc.gpsimd.indirect_dma_start(
                        out=hnt[:], out_offset=None, in_=h16[:],
                        in_offset=bass.IndirectOffsetOnAxis(ap=ninit_t[:, :1], axis=0))

                    # gather h_src for all edges (once per tile, reused 4 iters)
                    g = gsrcp.tile([P, C, F], b16, tag="g")
                    if SA:
                        nc.gpsimd.dma_gather(
                            out_ap=g[:, 0:SA, :], in_ap=h16[0:SPLIT, :], idxs_ap=idxA_t,
                            num_idxs=SA * P, num_idxs_reg=SA * P, elem_size=F, single_packet=False)
                    if SB:
                        nc.gpsimd.dma_gather(
                            out_ap=g[:, SA:C, :], in_ap=h16[SPLIT:NPAD, :], idxs_ap=idxB_t,
                            num_idxs=SB * P, num_idxs_reg=SB * P, elem_size=F, single_packet=False)

                    # edge-major onehot (for aggregation matmul)
                    onehot = ohp.tile([P, C, P], b16, tag="onehot")
                    nc.vector.tensor_tensor(
                        out=onehot[:],
                        in0=seg_t.unsqueeze(-1).to_broadcast([P, C, P]),
                        in1=iota_t.unsqueeze(1).to_broadcast([P, C, P]),
                        op=OP.is_equal)
                    # transposed onehot (for hd expansion matmul), via PE
                    onehotT = ohp.tile([P, C, P], b16, tag="onehotT")
                    for cb0 in range(0, C, CB):
                        nb = min(CB, C - cb0)
                        psTf = pswork.tile([P, CB, F], f32, tag="pse")
                        psT = psTf[:].bitcast(b16)
                        for c in range(nb):
                            nc.tensor.transpose(
                                out=psT[:, c, 0:P], in_=onehot[:, cb0 + c, :], identity=ident_t)
                        nc.scalar.activation(
                            out=onehotT[:, cb0:cb0 + nb, :], in_=psT[:, 0:nb, 0:P], func=AF.Copy)

                    st[j] = dict(C=C, g=g, onehot=onehot, onehotT=onehotT, hnt=hnt,
                                 hd=hnt)

                for it in range(ITERS):
                    last_it = it == ITERS - 1
                    # expansion + prod (per PSUM batch of CB chunks)
                    for j in grp:
                        C, g, onehotT, hd = (st[j][k] for k in ("C", "g", "onehotT", "hd"))
                        prod = hdp.tile([P, C, F], b16, tag="prod")
                        for cb0 in range(0, C, CB):
                            nb = min(CB, C - cb0)
                            pse = pswork.tile([P, CB, F], f32, tag="pse")
                            for c in range(nb):
                                nc.tensor.matmul(
                                    out=pse[:, c, :], lhsT=onehotT[:, cb0 + c, :], rhs=hd[:],
                                    start=True, stop=True)
                            hdx = hdp.tile([P, CB, F], b16, tag="hdx")
                            nc.scalar.activation(out=hdx[:, 0:nb, :], in_=pse[:, 0:nb, :], func=AF.Copy)
                            nc.vector.tensor_tensor(
                                out=prod[:, cb0:cb0 + nb, :], in0=g[:, cb0:cb0 + nb, :],
                                in1=hdx[:, 0:nb, :], op=OP.mult)
                        st[j]["prod"] = prod
                    # scores (reduce tree) + exp + weighted combo
                    for j in grp:
                        C, g, prod = st[j]["C"], st[j]["g"], st[j]["prod"]
                        pv = prod[:].rearrange("p c (d k) -> p c d k", d=D)
                        t1 = hdp.tile([P, C, 8, K], b16, tag="t1")
                        nc.vector.tensor_tensor(out=t1[:], in0=pv[:, :, 0:8, :], in1=pv[:, :, 8:16, :], op=OP.add)
                        t2 = hdp.tile([P, C, 4, K], b16, tag="t2")
                        nc.vector.tensor_tensor(out=t2[:], in0=t1[:, :, 0:4, :], in1=t1[:, :, 4:8, :], op=OP.add)
                        t3 = hdp.tile([P, C, 2, K], b16, tag="t3")
                        nc.vector.tensor_tensor(out=t3[:], in0=t2[:, :, 0:2, :], in1=t2[:, :, 2:4, :], op=OP.add)
                        scores = hdp.tile([P, C, K], f32, tag="scores")
                        nc.vector.tensor_tensor(
                            out=scores[:].unsqueeze(2), in0=t3[:, :, 0:1, :], in1=t3[:, :, 1:2, :], op=OP.add)
                        combo = hdp.tile([P, C, K + F], b16, tag="combo")
                        nc.scalar.activation(out=combo[:, :, 0:K], in_=scores[:], func=AF.Exp)
                        nc.vector.tensor_tensor(
                            out=combo[:, :, K:K + F].rearrange("p c (d k) -> p c d k", d=D),
                            in0=g[:].rearrange("p c (d k) -> p c d k", d=D),
                            in1=combo[:, :, 0:K].unsqueeze(2).to_broadcast([P, C, D, K]),
                            op=OP.mult)
                        st[j]["combo"] = combo
                    # aggregation matmul (PSUM-accumulated over chunks)
                    for j in grp:
                        C, onehot, combo = st[j]["C"], st[j]["onehot"], st[j]["combo"]
                        da = psdap.tile([P, K + F], f32, tag="da")
                        for c in range(C):
                            nc.tensor.matmul(out=da[:], lhsT=onehot[:, c, :], rhs=combo[:, c, :],
                                             start=(c == 0), stop=(c == C - 1))
                        st[j]["da"] = da
                    # epilogue: normalize (ln/exp batched per group), update hd
                    ss2g = smallp.tile([P, G, K], f32, tag="ss2g")
                    attrs = {}
                    for gj, j in enumerate(grp):
                        C, da, hnt = st[j]["C"], st[j]["da"], st[j]["hnt"]
                        deps = smallp.tile([P, K], f32, tag="deps")
                        nc.vector.tensor_scalar_add(deps[:], da[:, 0:K], 1e-6)
                        rden = smallp.tile([P, K], f32, tag="rden")
                        nc.vector.reciprocal(out=rden[:], in_=deps[:])
                        attr = smallp.tile([P, F], f32, tag="attr")
                        nc.vector.tensor_tensor(
                            out=attr[:].rearrange("p (d k) -> p d k", d=D),
                            in0=da[:, K:K + F].rearrange("p (d k) -> p d k", d=D),
                            in1=rden[:].unsqueeze(1).to_broadcast([P, D, K]),
                            op=OP.mult)
                        nc.vector.tensor_tensor(out=attr[:], in0=attr[:], in1=hnt[:], op=OP.add)
                        sq2 = smallp.tile([P, F], f32, tag="sq2")
                        nc.vector.tensor_tensor(out=sq2[:], in0=attr[:], in1=attr[:], op=OP.mult)
                        nc.vector.tensor_reduce(
                            out=ss2g[:, gj], in_=sq2[:].rearrange("p (d k) -> p k d", d=D),
                            axis=mybir.AxisListType.X, op=OP.add)
                        attrs[j] = attr
                    ng = len(grp)
                    ln2g = smallp.tile([P, G, K], f32, tag="ln2g")
                    nc.scalar.activation(out=ln2g[:, 0:ng], in_=ss2g[:, 0:ng], func=AF.Ln, bias=eps_t[:])
                    rs2g = smallp.tile([P, G, K], f32, tag="rs2g")
                    nc.scalar.activation(out=rs2g[:, 0:ng], in_=ln2g[:, 0:ng], func=AF.Exp, scale=-0.5)
                    for gj, j in enumerate(grp):
                        attr = attrs[j]
                        if not last_it:
                            hnew = smallp.tile([P, F], b16, tag="hnew")
                            nc.vector.tensor_tensor(
                                out=hnew[:].rearrange("p (d k) -> p d k", d=D),
                                in0=attr[:].rearrange("p (d k) -> p d k", d=D),
                                in1=rs2g[:, gj].unsqueeze(1).to_broadcast([P, D, K]),
                                op=OP.mult)
                            st[j]["hd"] = hnew
                        else:
                            hout = smallp.tile([P, F], f32, tag="hout")
                            nc.vector.tensor_tensor(
                                out=hout[:].rearrange("p (d k) -> p d k", d=D),
                                in0=attr[:].rearrange("p (d k) -> p d k", d=D),
                                in1=rs2g[:, gj].unsqueeze(1).to_broadcast([P, D, K]),
                                op=OP.mult)
                            nc.sync.dma_start(out=out[j * P:(j + 1) * P, :], in_=hout[:])
    if not nc.is_finalized():
        nc.finalize()
    return nc


# --------------------------------------------------------------------------
# entry point
# --------------------------------------------------------------------------

def kernel(x, edge_index, weight, bias):
    x = np.asarray(x, dtype=np.float32)
    weight = np.asarray(weight, dtype=np.float32)
    bias = np.asarray(bias, dtype=np.float32)
    assert x.shape == (N_NODES, F) and edge_index.shape == (2, N_EDGES)

    metas, offs, node_order, cA, cB = _preprocess(edge_index)
    with_bias = bool(np.any(bias != 0))
    nc = _build(offs, cA, cB, with_bias, metas[0].shape[1])

    # device uses (d, k)-interleaved feature order: f' = d*K + k <-> f = k*D + d
    perm = np.array([k * D + d for d in range(D) for k in range(K)])
    xpad = np.zeros((NPAD, F), np.float32)
    xpad[:N_NODES] = x
    xt = np.ascontiguousarray(xpad.T)                        # [128, NPAD] f32
    wp = np.ascontiguousarray(weight[:, perm])
    const = np.zeros((P, 512), np.int16)
    const[:, 0:128] = np.tile(np.arange(P, dtype=bf16)[None, :], (P, 1)).view(np.int16)
    const[:, 128:256] = np.eye(P, dtype=bf16).view(np.int16)
    const[:, 256:512] = np.tile(bias[perm].astype(np.float32)[None, :], (P, 1)).view(np.int16)

    in_maps = [
        dict(xt=xt, w=wp, meta=metas[c], const=const) for c in range(NCORES)
    ]
    res = run_bass_kernel_spmd(nc, in_maps, core_ids=list(range(NCORES)), trace=TRACE)
    LAST_RESULTS["exec_time_ns"] = res.exec_time_ns
    LAST_RESULTS["trace"] = res.instructions_and_trace
    LAST_RESULTS["nc"] = nc
    LAST_RESULTS["in_maps"] = in_maps

    full = np.zeros((NPAD, F), np.float32)
    for c in range(NCORES):
        full[node_order[c][:, None], perm[None, :]] = res.results[c]["out"]
    return full[:N_NODES]
